# revision 1
# baseline (speedup 1.0000x reference)
"""CanonCausalMultiheadAttn Trainium2 kernel.

Sharding: 8 cores = 2 (batch) x 4 (kv-head groups). Core c handles batch
c//4 and kv-group g=c%4 (q heads 4g..4g+3, kv head g). w_q/w_k/w_v are
column-sharded by head group, w_o row-sharded; each core emits a partial
[S, D] output which the host sums over the 4 groups of its batch.

Per-core dataflow (everything in transposed [feature, token] layout so no
on-chip transposes are needed except v):
  qkvT[f, t] = w_qkv[:, f].T @ hT[:, t]          (bf16 matmuls, fp32 PSUM)
  conv: per-partition-scalar taps along the free (token) axis + residual
  scores.T[k, q] = kT.T @ qT  -> exp (no max-sub needed; |s|<~8) -> probsT
  causal: only k-tiles with k0 <= q_end computed; diagonal tiles use a
  precomputed multiplicative triangular mask and a shrunken q-region.
  attT[dh, q] += v_nat[k,:].T @ probsT   (v_nat from 16 PE transposes)
  sums[*, q]  += ones128.T @ probsT      (softmax denominator, replicated
                                          across partitions so DVE can divide)
  out[t, d]  = attT.T @ w_o_rows         (partial; host sums over groups)
"""

import numpy as np
import ml_dtypes
from contextlib import ExitStack

import concourse.bass as bass
import concourse.tile as tile
import concourse.mybir as mybir
from concourse.bass import ds, ts
from concourse.bass_utils import run_bass_kernel_spmd
from concourse.masks import make_identity

BF16 = mybir.dt.bfloat16
F32 = mybir.dt.float32
P = 128
S = 2048          # sequence length
D = 2048          # d_model
NF = 6            # feature chunks of 128: 4 q heads, 1 k, 1 v
KT = D // P       # 16 contraction chunks over d_model
NQT = S // 512    # 4 query tiles of 512
NTT = S // 512    # 4 token tiles of 512
ISQ = 1.0 / np.sqrt(128.0)
MULT = mybir.AluOpType.mult
ADD = mybir.AluOpType.add

_CACHE = {}


def _legalize_waits(nc):
    """Split multi-wait sync_info into preceding single-wait engine NOPs.

    The walrus codegen in this container accepts at most ONE sync wait per
    TPB instruction ("Too many sync wait commands"), but the Tile scheduler
    freely emits several. An engine executes its queue in order, so hoisting
    the extra waits onto NoOps right before the instruction is equivalent.
    """
    n = 0
    for f in nc.m.functions:
        for blk in f.blocks:
            out = []
            changed = False
            for inst in blk.instructions:
                si = inst.sync_info
                if (si is not None and si.on_wait and len(si.on_wait) > 1
                        and str(inst.engine) != "EngineType.Unassigned"):
                    waits = list(si.on_wait)
                    for w in waits[:-1]:
                        out.append(mybir.InstNoOp(
                            name=f"I-wf{n}", engine=inst.engine, ins=[],
                            outs=[],
                            sync_info=mybir.SyncInfo(on_wait=[w],
                                                     on_update=[])))
                        n += 1
                    si.on_wait = [waits[-1]]
                    changed = True
                out.append(inst)
            if changed:
                blk.instructions = out
    return n


def _build():
    if "nc" in _CACHE:
        return _CACHE["nc"]
    nc = bass.Bass("TRN2", target_bir_lowering=False, debug=False)

    hT_d = nc.dram_tensor("hT", [D, S], BF16, kind="ExternalInput").ap()
    wqkv_d = nc.dram_tensor("w_qkv", [D, NF * P], BF16, kind="ExternalInput").ap()
    wo_d = nc.dram_tensor("w_o", [4 * P, D], BF16, kind="ExternalInput").ap()
    cw_d = nc.dram_tensor("conv_w", [P, NF * 4], F32, kind="ExternalInput").ap()
    out_d = nc.dram_tensor("out", [S, D], F32, kind="ExternalOutput").ap()

    hT_v = hT_d.rearrange("(ko ki) t -> ki ko t", ki=P)        # [128,16,2048]
    wqkv_v = wqkv_d.rearrange("(ko ki) f -> ki ko f", ki=P)    # [128,16,768]
    wo_v = wo_d.rearrange("(c ki) d -> ki c d", ki=P)          # [128,4,2048]
    out_v = out_d.rearrange("(po pi) d -> pi po d", pi=P)      # [128,16,2048]

    with tile.TileContext(nc) as tc, ExitStack() as ctx:
        const = ctx.enter_context(tc.tile_pool(name="const", bufs=1))
        p_ht = ctx.enter_context(tc.tile_pool(name="ht", bufs=2))
        p_work = ctx.enter_context(tc.tile_pool(name="work", bufs=2))
        p_probs = ctx.enter_context(tc.tile_pool(name="probs", bufs=3))
        p_out = ctx.enter_context(tc.tile_pool(name="outp", bufs=6))
        ps2 = ctx.enter_context(tc.tile_pool(name="ps2", bufs=2, space="PSUM"))
        ps3 = ctx.enter_context(tc.tile_pool(name="ps3", bufs=3, space="PSUM"))
        ps1 = ctx.enter_context(tc.tile_pool(name="ps1", bufs=1, space="PSUM"))

        # --- constants / persistent tensors ---
        ident = const.tile([P, P], BF16, tag="ident")
        make_identity(nc, ident)
        # mask[k, x] = 1.0 if x >= k else 0.0 (shared by all diagonal tiles)
        mask = const.tile([P, 512], BF16, tag="mask")
        nc.gpsimd.memset(mask, 1.0)
        nc.gpsimd.affine_select(
            out=mask, in_=mask, pattern=[[1, 512]], base=0,
            channel_multiplier=-1, compare_op=mybir.AluOpType.is_ge, fill=0.0,
        )
        cw0 = const.tile([P, NF * 4], F32, tag="cw0")
        nc.sync.dma_start(cw0, cw_d)
        # conv ops read cw via a DVE copy so their DMA wait lands here, not
        # on the (wait-slot-limited) scalar_tensor_tensor instructions
        cw = const.tile([P, NF * 4], F32, tag="cw")
        nc.vector.tensor_copy(cw, cw0)
        wq_sb = const.tile([P, KT, NF * P], BF16, tag="wq")
        wo_sb = const.tile([P, 4, D], BF16, tag="wo")
        # raw (pre-conv) qkv.T in fp32, with 3 leading zero columns so the
        # causal conv taps can read t-3..t-1 without edge cases
        qkvf = const.tile([P, NF, S + 4], F32, tag="qkvf")
        # zero the pad on ACT so conv's read-waits coalesce with the ACT
        # projection copies (one sem instead of two)
        nc.scalar.memzero(qkvf[:, :, 0:4])
        qkvb = const.tile([P, NF, S], BF16, tag="qkvb")    # conv'd qkv.T (bf16)
        vnat = const.tile([P, KT, P], BF16, tag="vnat")    # v in [token, dh]
        attT = const.tile([P, 4, S], BF16, tag="attT")     # attended.T per head

        FP32R = mybir.dt.float32r
        ones_f = const.tile([P, P], F32, tag="ones_f")
        nc.vector.memset(ones_f, 1.0)
        ones_r = const.tile([P, P], FP32R, tag="ones_r")
        nc.vector.tensor_copy(ones_r, ones_f)

        def o_proj_chunk(qt, t4, tags=("proj",)):
            # output projection for one token-128-tile of q-tile qt
            tt16 = qt * 4 + t4
            for dt in range(4):
                op = ps2.tile([P, 512], F32, tag=tags[dt % len(tags)])
                for fc4 in range(4):
                    nc.tensor.matmul(
                        op, lhsT=attT[:, fc4, ds(tt16 * P, P)],
                        rhs=wo_sb[:, fc4, ds(dt * 512, 512)],
                        start=(fc4 == 0), stop=(fc4 == 3))
                ob = p_out.tile([P, 512], F32, tag="ob")
                nc.scalar.copy(ob, op)
                nc.sync.dma_start(out_v[:, tt16, ds(dt * 512, 512)], ob)

        def attn_B(qt):
            # attention for q-tile qt (needs phase A token tiles <= qt);
            # the previous q-tile's output projection is interleaved per-head
            # as PE filler while ACT/DVE work through exp/softmax chains.
            for h in range(4):
                nk = 4 * (qt + 1)
                att = ps3.tile([P, 512], F32, tag="att")
                colsum = p_work.tile([P, 512], FP32R, tag="colsum")
                prev = None
                prev2 = None
                pr_quad = None
                for kt in range(nk + 2):
                    if kt == min(4, nk - 2) and qt > 0:
                        # previous q-tile's output projection emitted mid-head:
                        # PE filler while ACT/DVE chew exp + softmax chains
                        o_proj_chunk(qt - 1, h)
                    if kt < nk:
                        j = kt - 4 * qt
                        x0 = j * P if j >= 0 else 0
                        F = 512 - x0
                        sp = ps2.tile([P, 512], F32, tag="s")
                        nc.tensor.matmul(
                            sp[:, x0:512],
                            lhsT=qkvb[:, 4, ds(kt * P, P)],
                            rhs=qkvb[:, h, ds(qt * 512 + x0, F)],
                            start=True, stop=True,
                        )
                        if kt % 4 == 0:
                            pr_quad = p_probs.tile([P, 4, 512], BF16,
                                                   tag="probs")
                        pr = pr_quad[:, kt % 4, :]
                        nc.scalar.activation(
                            pr[:, x0:512], sp[:, x0:512],
                            mybir.ActivationFunctionType.Exp, scale=ISQ)
                        if j >= 0:
                            nc.vector.tensor_mul(
                                pr[:, x0:512], pr[:, x0:512], mask[:, 0:F])
                        cur = (pr, x0, kt)
                    else:
                        cur = None
                    if prev2 is not None:
                        ppr, px0, pkt = prev2
                        nc.tensor.matmul(
                            att[:, px0:512], lhsT=vnat[:, pkt, :],
                            rhs=ppr[:, px0:512],
                            start=(pkt == 0), stop=(pkt == nk - 1))
                        # softmax denominator: accumulate exp'd probs on DVE
                        # (partition dim reduced by ONE ones-matmul at the end)
                        if pkt == 0:
                            nc.vector.tensor_copy(colsum, ppr)
                        else:
                            nc.vector.tensor_add(
                                colsum[:, px0:512], colsum[:, px0:512],
                                ppr[:, px0:512])
                    prev2 = prev
                    prev = cur
                smp = ps1.tile([P, 512], F32, tag="small")
                nc.tensor.matmul(smp, lhsT=ones_r, rhs=colsum,
                                 start=True, stop=True)
                rec = p_work.tile([P, 512], F32, tag="rec")
                nc.vector.reciprocal(rec, smp)
                nc.vector.tensor_mul(attT[:, h, ts(qt, 512)], att, rec)

        # ------- Fused phases: per token tile: projection+conv, then the
        # attention q-tile that just became computable, then the (pipelined)
        # output projection of the previous q-tile. Keeps PE dense while
        # spreading ACT(exp)/DVE(softmax) work across the whole timeline.
        for tt in range(NTT):
            ht = p_ht.tile([P, KT, 512], BF16, tag="ht")
            for k2 in range(8):
                # pair-chunk DMAs: fewer dispatches than per-chunk, still
                # fine-grained enough that the first matmuls start early
                if tt == 0:
                    nc.sync.dma_start(wq_sb[:, ds(k2 * 2, 2), :],
                                      wqkv_v[:, ds(k2 * 2, 2), :])
                nc.sync.dma_start(ht[:, ds(k2 * 2, 2), :],
                                  hT_v[:, ds(k2 * 2, 2), ts(tt, 512)])
            t0 = tt * 512

            def conv(fc):
                # conv taps: out[t] = x[t] + sum_k x[t+k-3]*w[k]
                tmp = p_work.tile([P, 512], F32, tag="ctmp", name="ctmp")
                nc.vector.scalar_tensor_tensor(
                    tmp, qkvf[:, fc, ds(t0 + 0, 512)],
                    cw[:, fc * 4 + 0: fc * 4 + 1],
                    qkvf[:, fc, ds(t0 + 3, 512)], op0=MULT, op1=ADD)
                nc.vector.scalar_tensor_tensor(
                    tmp, qkvf[:, fc, ds(t0 + 1, 512)],
                    cw[:, fc * 4 + 1: fc * 4 + 2], tmp, op0=MULT, op1=ADD)
                nc.vector.scalar_tensor_tensor(
                    tmp, qkvf[:, fc, ds(t0 + 2, 512)],
                    cw[:, fc * 4 + 2: fc * 4 + 3], tmp, op0=MULT, op1=ADD)
                nc.vector.scalar_tensor_tensor(
                    qkvb[:, fc, ts(tt, 512)], qkvf[:, fc, ds(t0 + 3, 512)],
                    cw[:, fc * 4 + 3: fc * 4 + 4], tmp, op0=MULT, op1=ADD)

            if tt == 0:
                # split each fc group into two 8-chunk halves, all A-halves
                # first: unblocks 48 matmuls once half the head DMA burst has
                # landed instead of stalling on the last chunk
                for fc in range(NF):
                    pp = ps2.tile([P, 512], F32, tag="proj", name="pp")
                    for kk in range(8):
                        nc.tensor.matmul(
                            pp, lhsT=wq_sb[:, kk, ds(fc * P, P)],
                            rhs=ht[:, kk, :],
                            start=(kk == 0), stop=(kk == 7))
                    nc.scalar.copy(qkvf[:, fc, ds(3, 512)], pp)
                for fc in range(NF):
                    pp = ps2.tile([P, 512], F32, tag="proj", name="pp")
                    for kk in range(8, KT):
                        nc.tensor.matmul(
                            pp, lhsT=wq_sb[:, kk, ds(fc * P, P)],
                            rhs=ht[:, kk, :],
                            start=(kk == 8), stop=(kk == KT - 1))
                    nc.vector.tensor_add(
                        qkvf[:, fc, ds(3, 512)], qkvf[:, fc, ds(3, 512)], pp)
                    conv(fc)
            else:
                for fc in range(NF):
                    pp = ps2.tile([P, 512], F32, tag="proj", name="pp")
                    for kk in range(KT):
                        nc.tensor.matmul(
                            pp, lhsT=wq_sb[:, kk, ds(fc * P, P)],
                            rhs=ht[:, kk, :],
                            start=(kk == 0), stop=(kk == KT - 1),
                        )
                    nc.scalar.copy(qkvf[:, fc, ds(3 + t0, 512)], pp)
                    conv(fc)
            # v (fc=5) of this token tile -> natural [token, dh] layout
            for j in range(4):
                kt_i = tt * 4 + j
                trp = ps1.tile([P, 512], BF16, tag="small")
                nc.tensor.transpose(trp[:, 0:P], qkvb[:, 5, ds(kt_i * P, P)],
                                    ident)
                nc.vector.tensor_copy(vnat[:, kt_i, :], trp[:, 0:P])
            if tt == 0:
                # w_o load deferred past the critical head DMAs
                nc.sync.dma_start(wo_sb, wo_v)
            attn_B(tt)
        for t4 in range(4):
            o_proj_chunk(NQT - 1, t4, tags=("proj", "s"))

    _legalize_waits(nc)
    _CACHE["nc"] = nc
    return nc


def _prep_inputs(hidden_states, w_q, w_k, w_v, w_o, conv_w):
    """Build the 8 per-core input maps (host-side shard + bf16 cast)."""
    bf = ml_dtypes.bfloat16
    in_maps = []
    for c in range(8):
        b, g = c // 4, c % 4
        hT = np.ascontiguousarray(hidden_states[b].T).astype(bf)
        wqkv = np.concatenate(
            [w_q[:, g * 512:(g + 1) * 512],
             w_k[:, g * 128:(g + 1) * 128],
             w_v[:, g * 128:(g + 1) * 128]], axis=1).astype(bf)
        wo = np.ascontiguousarray(w_o[g * 512:(g + 1) * 512, :]).astype(bf)
        cw = np.concatenate(
            [conv_w[g * 512:(g + 1) * 512],
             conv_w[2048 + g * 128: 2048 + (g + 1) * 128],
             conv_w[2560 + g * 128: 2560 + (g + 1) * 128]], axis=0)  # [768,4]
        cw = np.ascontiguousarray(
            cw.reshape(NF, P, 4).transpose(1, 0, 2).reshape(P, NF * 4)
        ).astype(np.float32)
        in_maps.append({"hT": hT, "w_qkv": wqkv, "w_o": wo, "conv_w": cw})
    return in_maps


def kernel(hidden_states, w_q, w_k, w_v, w_o, conv_w, _trace=False):
    nc = _build()
    in_maps = _prep_inputs(
        np.asarray(hidden_states, dtype=np.float32),
        np.asarray(w_q, dtype=np.float32),
        np.asarray(w_k, dtype=np.float32),
        np.asarray(w_v, dtype=np.float32),
        np.asarray(w_o, dtype=np.float32),
        np.asarray(conv_w, dtype=np.float32),
    )
    res = run_bass_kernel_spmd(nc, in_maps, core_ids=list(range(8)),
                               trace=_trace)
    outs = [r["out"] for r in res.results]
    full = np.empty((2, S, D), dtype=np.float32)
    for b in range(2):
        full[b] = outs[4 * b] + outs[4 * b + 1] + outs[4 * b + 2] + outs[4 * b + 3]
    if _trace:
        kernel.last_results = res
    return full



# revision 17
# speedup vs baseline: 1.0434x; 1.0434x over previous
"""CanonCausalMultiheadAttn Trainium2 kernel (fp8 DoubleRow version).

Sharding: 8 cores = 2 (batch) x 4 (kv-head groups). Core c handles batch
c//4 and kv-group g=c%4 (q heads 4g..4g+3, kv head g). w_q/w_k/w_v are
column-sharded by head group, w_o row-sharded; each core emits a partial
[S, D] output which the host sums over the 4 groups of its batch.

Per-core dataflow (transposed [feature, token] layout; v transposed on PE):
  qkvT[f, t] = w_qkv[:, f].T @ hT[:, t]   -- fp8e4m3 DoubleRow matmuls with
      3-term hi/lo compensation (w_hi.h_hi + w_lo.h_hi + w_hi.h_lo), which
      matches bf16 accuracy at 0.75x the PE time (DR = 0.5 cy/row, 256-deep
      contraction per instruction).
  conv: depthwise causal taps in bf16 on DVE (per-partition-scalar
      mult-add along the free axis); weights stay near-exact.
  scores.T[k, q] = kT.T @ qT (bf16) -> exp on ACT (scale folds the fp8
      pre-scales; bias -2ln2 keeps fp16 column sums in range)
  causal: k-tiles with k0 <= q_end only; diagonal tiles masked in-place
      by an affine_select on the (otherwise idle) Pool engine.
  attT[dh, q] += v_nat[k,:].T @ probsT  (bf16)
  colsum via DVE adds in fp16 (2x DVE mode), partition-reduced by one
      fp16 ones-matmul (ones=4.0 folds the attT scale correction).
  attT stored as fp8 hi+lo; out[t, d] = attT.T @ w_o_rows via 3-term DR.
  Host divides the gathered partial sums by the net 2048x scale.

Scales: w_qkv and w_o are pre-scaled x64 into fp8 (e4m3 = IEEE variant,
  max finite 240); hidden stays x1. qkv = 64x, scores = 4096 s (folded
  into exp scale), probs = p/4 (exp bias), att = 16*Sum p v, colsum = p/4
  summed, ones=4.0 => attT = 16*attended, out = 1024*true. Host divides.
"""

import numpy as np
import ml_dtypes
from contextlib import ExitStack

import concourse.bass as bass
import concourse.tile as tile
import concourse.mybir as mybir
from concourse.bass import ds, ts
from concourse.bass_utils import run_bass_kernel_spmd
from concourse.masks import make_identity

BF16 = mybir.dt.bfloat16
F16 = mybir.dt.float16
F32 = mybir.dt.float32
FP8 = mybir.dt.float8e4
DR = mybir.MatmulPerfMode.DoubleRow
P = 128
S = 2048          # sequence length
D = 2048          # d_model
NF = 6            # feature chunks of 128: 4 q heads, 1 k, 1 v
NPR = 8           # DR contraction pairs over d_model (2048 = 8*256)
NQT = S // 512    # 4 query tiles of 512
NTT = S // 512    # 4 token tiles of 512
WSCALE = 64.0     # fp8 pre-scale on w_qkv and w_o (e4m3 max is 240)
ISQ = 1.0 / np.sqrt(128.0)
EXP_SCALE = ISQ / (WSCALE * WSCALE)   # scores PSUM holds 4096*s
EXP_BIAS = float(-2.0 * np.log(2.0))  # probs = p/4 (fp16 colsum headroom)
OUT_DIV = 1024.0  # 16 (attT) * 64 (w_o)
MULT = mybir.AluOpType.mult
ADD = mybir.AluOpType.add

_CACHE = {}


def _legalize_waits(nc):
    """Split multi-wait sync_info into preceding single-wait engine NOPs.

    The walrus codegen in this container accepts at most ONE sync wait per
    TPB instruction ("Too many sync wait commands"), but the Tile scheduler
    freely emits several. An engine executes its queue in order, so hoisting
    the extra waits onto NoOps right before the instruction is equivalent.
    """
    n = 0
    for f in nc.m.functions:
        for blk in f.blocks:
            out = []
            changed = False
            for inst in blk.instructions:
                si = inst.sync_info
                if (si is not None and si.on_wait and len(si.on_wait) > 1
                        and str(inst.engine) != "EngineType.Unassigned"):
                    waits = list(si.on_wait)
                    for w in waits[:-1]:
                        out.append(mybir.InstNoOp(
                            name=f"I-wf{n}", engine=inst.engine, ins=[],
                            outs=[],
                            sync_info=mybir.SyncInfo(on_wait=[w],
                                                     on_update=[])))
                        n += 1
                    si.on_wait = [waits[-1]]
                    changed = True
                out.append(inst)
            if changed:
                blk.instructions = out
    return n


def _build(legalize=True, debug=False):
    key = ("nc" if legalize else "nc_raw") + ("_dbg" if debug else "")
    if key in _CACHE:
        return _CACHE[key]
    nc = bass.Bass("TRN2", target_bir_lowering=False, debug=False)

    hh_d = nc.dram_tensor("h_hi", [P, NPR, 2, S], FP8, kind="ExternalInput").ap()
    hl_d = nc.dram_tensor("h_lo", [P, NPR, 2, S], FP8, kind="ExternalInput").ap()
    wh_d = nc.dram_tensor("w_hi", [P, NPR, 2, NF * P], FP8, kind="ExternalInput").ap()
    wl_d = nc.dram_tensor("w_lo", [P, NPR, 2, NF * P], FP8, kind="ExternalInput").ap()
    oh_d = nc.dram_tensor("wo_hi", [P, 2, 2, D], FP8, kind="ExternalInput").ap()
    ol_d = nc.dram_tensor("wo_lo", [P, 2, 2, D], FP8, kind="ExternalInput").ap()
    cw_d = nc.dram_tensor("conv_w", [P, NF * 4], F32, kind="ExternalInput").ap()
    out_d = nc.dram_tensor("out", [S, D], F32, kind="ExternalOutput").ap()
    if debug:
        dbg_qkvb = nc.dram_tensor("dbg_qkvb", [P, NF, S], BF16,
                                  kind="ExternalOutput").ap()
        dbg_atth = nc.dram_tensor("dbg_atth", [P, 4, S], FP8,
                                  kind="ExternalOutput").ap()
        dbg_qkvf8 = nc.dram_tensor("dbg_qkvf8", [P, NF, S + 3], BF16,
                                   kind="ExternalOutput").ap()

    out_v = out_d.rearrange("(po pi) d -> pi po d", pi=P)      # [128,16,2048]

    with tile.TileContext(nc) as tc, ExitStack() as ctx:
        const = ctx.enter_context(tc.tile_pool(name="const", bufs=1))
        p_ht = ctx.enter_context(tc.tile_pool(name="ht", bufs=2))
        p_work = ctx.enter_context(tc.tile_pool(name="work", bufs=2))
        p_probs = ctx.enter_context(tc.tile_pool(name="probs", bufs=3))
        p_out = ctx.enter_context(tc.tile_pool(name="outp", bufs=6))
        ps2 = ctx.enter_context(tc.tile_pool(name="ps2", bufs=2, space="PSUM"))
        ps3 = ctx.enter_context(tc.tile_pool(name="ps3", bufs=3, space="PSUM"))
        ps1 = ctx.enter_context(tc.tile_pool(name="ps1", bufs=1, space="PSUM"))

        # --- constants / persistent tensors ---
        ident = const.tile([P, P], BF16, tag="ident")
        make_identity(nc, ident)
        cw0 = const.tile([P, NF * 4], F32, tag="cw0")
        nc.sync.dma_start(cw0, cw_d)
        # conv ops read cw via a DVE copy so their DMA wait lands here, not
        # on the (wait-slot-limited) Pool scalar_tensor_tensor instructions
        cw = const.tile([P, NF * 4], F32, tag="cw")
        nc.vector.tensor_copy(cw, cw0)
        wq_hi = const.tile([P, NPR, 2, NF * P], FP8, tag="wqh")
        wq_lo = const.tile([P, NPR, 2, NF * P], FP8, tag="wql")
        wo_hi = const.tile([P, 2, 2, D], FP8, tag="woh")
        wo_lo = const.tile([P, 2, 2, D], FP8, tag="wol")
        # raw (pre-conv) qkv.T in bf16 (64x scale), 3 leading zero columns so
        # the causal conv taps can read t-3..t-1 without edge cases
        qkvf = const.tile([P, NF, S + 3], BF16, tag="qkvf")
        nc.gpsimd.memset(qkvf[:, :, 0:3], 0.0)
        qkvb = const.tile([P, NF, S], BF16, tag="qkvb")    # conv'd qkv.T
        vnat = const.tile([P, 16, P], BF16, tag="vnat")    # v in [token, dh]
        atth = const.tile([P, 4, S], FP8, tag="atth")      # attT hi per head
        attl = const.tile([P, 4, S], FP8, tag="attl")      # attT lo per head
        ones2 = const.tile([P, P], F16, tag="ones2")
        nc.vector.memset(ones2, 4.0)
        ebias = const.tile([P, 1], F32, tag="ebias")
        nc.vector.memset(ebias, EXP_BIAS)

        def o_proj_chunk(qt, t4, tags=("proj",)):
            # output projection for one token-128-tile of q-tile qt
            tt16 = qt * 4 + t4
            for dt in range(4):
                op = ps2.tile([P, 512], F32, tag=tags[dt % len(tags)])
                k = 0
                for lhs, rhs_w in ((atth, wo_hi), (attl, wo_hi), (atth, wo_lo)):
                    for pr_ in range(2):
                        nc.tensor.matmul(
                            op,
                            lhsT=lhs[:, ds(2 * pr_, 2), ds(tt16 * P, P)],
                            rhs=rhs_w[:, pr_, :, ds(dt * 512, 512)],
                            start=(k == 0), stop=(k == 5), perf_mode=DR)
                        k += 1
                ob = p_out.tile([P, 512], F32, tag="ob")
                nc.scalar.copy(ob, op)
                nc.sync.dma_start(out_v[:, tt16, ds(dt * 512, 512)], ob)

        def attn_B(qt):
            # attention for q-tile qt (needs token tiles <= qt); the previous
            # q-tile's output projection is interleaved per-head as PE filler
            # while ACT/DVE work through exp/softmax chains.
            for h in range(4):
                nk = 4 * (qt + 1)
                att = ps3.tile([P, 512], F32, tag="att")
                colsum = p_work.tile([P, 512], F16, tag="colsum")
                prev = None
                prev2 = None
                pr_quad = None
                for kt in range(nk + 2):
                    if kt == min(4, nk - 2) and qt > 0:
                        # previous q-tile's output projection emitted mid-head:
                        # PE filler while ACT/DVE chew exp + softmax chains
                        o_proj_chunk(qt - 1, h)
                    if kt < nk:
                        j = kt - 4 * qt
                        x0 = j * P if j >= 0 else 0
                        F = 512 - x0
                        sp = ps2.tile([P, 512], F32, tag="s")
                        nc.tensor.matmul(
                            sp[:, x0:512],
                            lhsT=qkvb[:, 4, ds(kt * P, P)],
                            rhs=qkvb[:, h, ds(qt * 512 + x0, F)],
                            start=True, stop=True,
                        )
                        if kt % 4 == 0:
                            pr_quad = p_probs.tile([P, 4, 512], BF16,
                                                   tag="probs")
                        pr = pr_quad[:, kt % 4, :]
                        nc.scalar.activation(
                            pr[:, x0:512], sp[:, x0:512],
                            mybir.ActivationFunctionType.Exp,
                            scale=EXP_SCALE, bias=ebias)
                        if j >= 0:
                            # zero the k>q half of the diagonal tile in place
                            # (local col c vs partition p: keep iff c >= p)
                            nc.gpsimd.affine_select(
                                out=pr[:, x0:512], in_=pr[:, x0:512],
                                pattern=[[1, F]], base=0,
                                channel_multiplier=-1,
                                compare_op=mybir.AluOpType.is_ge, fill=0.0)
                        cur = (pr, x0, kt)
                    else:
                        cur = None
                    if prev2 is not None:
                        ppr, px0, pkt = prev2
                        nc.tensor.matmul(
                            att[:, px0:512], lhsT=vnat[:, pkt, :],
                            rhs=ppr[:, px0:512],
                            start=(pkt == 0), stop=(pkt == nk - 1))
                        # softmax denominator: accumulate exp'd probs on DVE
                        # (partition dim reduced by ONE ones-matmul at the end)
                        if pkt == 0:
                            nc.vector.tensor_copy(colsum, ppr)
                        else:
                            nc.vector.tensor_add(
                                colsum[:, px0:512], colsum[:, px0:512],
                                ppr[:, px0:512])
                    prev2 = prev
                    prev = cur
                smp = ps1.tile([P, 512], F32, tag="small")
                nc.tensor.matmul(smp, lhsT=ones2, rhs=colsum,
                                 start=True, stop=True)
                rec = p_work.tile([P, 512], F32, tag="rec")
                nc.vector.reciprocal(rec, smp)
                t16 = p_work.tile([P, 512], F16, tag="t16")
                nc.vector.tensor_mul(t16, att, rec)
                nc.vector.tensor_copy(atth[:, h, ts(qt, 512)], t16)
                nc.vector.tensor_sub(attl[:, h, ts(qt, 512)], t16,
                                     atth[:, h, ts(qt, 512)])

        # ------- Fused phases: per token tile: projection+conv, then the
        # attention q-tile that just became computable, then the (pipelined)
        # output projection of the previous q-tile.
        for tt in range(NTT):
            ht_hi = p_ht.tile([P, NPR, 2, 512], FP8, tag="hth")
            ht_lo = p_ht.tile([P, NPR, 2, 512], FP8, tag="htl")
            for k2 in range(4):
                # pair-chunk DMAs; hi chunks (and weights) first so the hi*hi
                # chains can start early, lo afterwards
                if tt == 0:
                    nc.sync.dma_start(wq_hi[:, ds(k2 * 2, 2)],
                                      wh_d[:, ds(k2 * 2, 2)])
                nc.sync.dma_start(ht_hi[:, ds(k2 * 2, 2)],
                                  hh_d[:, ds(k2 * 2, 2), :, ts(tt, 512)])
            for k2 in range(4):
                if tt == 0:
                    nc.sync.dma_start(wq_lo[:, ds(k2 * 2, 2)],
                                      wl_d[:, ds(k2 * 2, 2)])
                nc.sync.dma_start(ht_lo[:, ds(k2 * 2, 2)],
                                  hl_d[:, ds(k2 * 2, 2), :, ts(tt, 512)])
            t0 = tt * 512

            for fc in range(NF):
                pp = ps2.tile([P, 512], F32, tag="proj", name="pp")
                k = 0
                for lhs, rhs_h in ((wq_hi, ht_hi), (wq_lo, ht_hi),
                                   (wq_hi, ht_lo)):
                    for pr_ in range(NPR):
                        nc.tensor.matmul(
                            pp, lhsT=lhs[:, pr_, :, ds(fc * P, P)],
                            rhs=rhs_h[:, pr_],
                            start=(k == 0), stop=(k == 3 * NPR - 1),
                            perf_mode=DR)
                        k += 1
                # pre-conv x (64x) -> bf16 for the Pool conv taps
                nc.scalar.copy(qkvf[:, fc, ds(3 + t0, 512)], pp)
                # conv taps: out[t] = x[t] + sum_k x[t+k-3]*w[k]
                tmp = p_work.tile([P, 512], BF16, tag="ctmp", name="ctmp")
                nc.vector.scalar_tensor_tensor(
                    tmp, qkvf[:, fc, ds(t0 + 0, 512)],
                    cw[:, fc * 4 + 0: fc * 4 + 1],
                    qkvf[:, fc, ds(t0 + 3, 512)], op0=MULT, op1=ADD)
                nc.vector.scalar_tensor_tensor(
                    tmp, qkvf[:, fc, ds(t0 + 1, 512)],
                    cw[:, fc * 4 + 1: fc * 4 + 2], tmp, op0=MULT, op1=ADD)
                nc.vector.scalar_tensor_tensor(
                    tmp, qkvf[:, fc, ds(t0 + 2, 512)],
                    cw[:, fc * 4 + 2: fc * 4 + 3], tmp, op0=MULT, op1=ADD)
                nc.vector.scalar_tensor_tensor(
                    qkvb[:, fc, ts(tt, 512)], qkvf[:, fc, ds(t0 + 3, 512)],
                    cw[:, fc * 4 + 3: fc * 4 + 4], tmp, op0=MULT, op1=ADD)
            # v (fc=5) of this token tile -> natural [token, dh] layout
            for j in range(4):
                kt_i = tt * 4 + j
                trp = ps1.tile([P, 512], BF16, tag="small")
                nc.tensor.transpose(trp[:, 0:P], qkvb[:, 5, ds(kt_i * P, P)],
                                    ident)
                nc.vector.tensor_copy(vnat[:, kt_i, :], trp[:, 0:P])
            if tt == 0:
                # w_o load deferred past the critical head DMAs
                nc.sync.dma_start(wo_hi, oh_d)
                nc.sync.dma_start(wo_lo, ol_d)
            attn_B(tt)
        for t4 in range(4):
            o_proj_chunk(NQT - 1, t4, tags=("proj", "s"))
        if debug:
            nc.sync.dma_start(dbg_qkvb, qkvb)
            nc.sync.dma_start(dbg_atth, atth)
            nc.sync.dma_start(dbg_qkvf8, qkvf)

    if legalize:
        _legalize_waits(nc)
    _CACHE[key] = nc
    return nc


def _prep_inputs(hidden_states, w_q, w_k, w_v, w_o, conv_w):
    """Build the 8 per-core input maps (host-side shard + fp8 hi/lo split)."""
    f8 = ml_dtypes.float8_e4m3

    def pairs(x, free):  # [2048, free] -> [128, 8, 2, free]
        return np.ascontiguousarray(
            x.reshape(NPR, 2, P, free).transpose(2, 0, 1, 3))

    def split8(x):
        hi = x.astype(f8)
        lo = (x - hi.astype(np.float32)).astype(f8)
        return hi, lo

    # hidden split is shared by the 4 cores of a batch
    h_pairs = []
    for b in range(2):
        hT = np.ascontiguousarray(hidden_states[b].T)
        hi, lo = split8(hT)
        h_pairs.append((pairs(hi, S), pairs(lo, S)))

    in_maps = []
    for c in range(8):
        b, g = c // 4, c % 4
        wqkv = np.concatenate(
            [w_q[:, g * 512:(g + 1) * 512],
             w_k[:, g * 128:(g + 1) * 128],
             w_v[:, g * 128:(g + 1) * 128]], axis=1) * WSCALE
        w_hi, w_lo = split8(wqkv)
        wo = np.ascontiguousarray(w_o[g * 512:(g + 1) * 512, :]) * WSCALE
        wo_hi, wo_lo = split8(wo)
        wo_hi = np.ascontiguousarray(
            wo_hi.reshape(2, 2, P, D).transpose(2, 0, 1, 3))
        wo_lo = np.ascontiguousarray(
            wo_lo.reshape(2, 2, P, D).transpose(2, 0, 1, 3))
        cw = np.concatenate(
            [conv_w[g * 512:(g + 1) * 512],
             conv_w[2048 + g * 128: 2048 + (g + 1) * 128],
             conv_w[2560 + g * 128: 2560 + (g + 1) * 128]], axis=0)  # [768,4]
        cwp = np.ascontiguousarray(
            cw.reshape(NF, P, 4).transpose(1, 0, 2).reshape(P, NF * 4)
        ).astype(np.float32)
        in_maps.append({
            "h_hi": h_pairs[b][0], "h_lo": h_pairs[b][1],
            "w_hi": pairs(w_hi, NF * P), "w_lo": pairs(w_lo, NF * P),
            "wo_hi": wo_hi, "wo_lo": wo_lo,
            "conv_w": cwp,
        })
    return in_maps


def kernel(hidden_states, w_q, w_k, w_v, w_o, conv_w, _trace=False):
    nc = _build()
    in_maps = _prep_inputs(
        np.asarray(hidden_states, dtype=np.float32),
        np.asarray(w_q, dtype=np.float32),
        np.asarray(w_k, dtype=np.float32),
        np.asarray(w_v, dtype=np.float32),
        np.asarray(w_o, dtype=np.float32),
        np.asarray(conv_w, dtype=np.float32),
    )
    res = run_bass_kernel_spmd(nc, in_maps, core_ids=list(range(8)),
                               trace=_trace)
    outs = [r["out"] for r in res.results]
    full = np.empty((2, S, D), dtype=np.float32)
    for b in range(2):
        acc = outs[4 * b] + outs[4 * b + 1] + outs[4 * b + 2] + outs[4 * b + 3]
        full[b] = acc * (1.0 / OUT_DIV)
    if _trace:
        kernel.last_results = res
    return full


# revision 25
# speedup vs baseline: 1.1349x; 1.0877x over previous
"""CanonCausalMultiheadAttn Trainium2 kernel (fp8 DoubleRow version).

Sharding: 8 cores = 2 (batch) x 4 (kv-head groups). Core c handles batch
c//4 and kv-group g=c%4 (q heads 4g..4g+3, kv head g). w_q/w_k/w_v are
column-sharded by head group, w_o row-sharded; each core emits a partial
[S, D] output which the host sums over the 4 groups of its batch.

Per-core dataflow (transposed [feature, token] layout; v transposed on PE):
  qkvT[f, t] = w_qkv[:, f].T @ hT[:, t]   -- fp8e4m3 DoubleRow matmuls with
      3-term hi/lo compensation (w_hi.h_hi + w_lo.h_hi + w_hi.h_lo), which
      matches bf16 accuracy at 0.75x the PE time (DR = 0.5 cy/row, 256-deep
      contraction per instruction).
  conv: depthwise causal taps in bf16 on DVE (per-partition-scalar
      mult-add along the free axis); weights stay near-exact.
  scores.T[k, q] = kT.T @ qT (bf16) -> exp on ACT (scale folds the fp8
      pre-scales; bias -2ln2 keeps fp16 column sums in range)
  causal: k-tiles with k0 <= q_end only; diagonal tiles masked in-place
      by an affine_select on the (otherwise idle) Pool engine.
  attT[dh, q] += v_nat[k,:].T @ probsT  (bf16)
  colsum via DVE adds in fp16 (2x DVE mode), partition-reduced by one
      fp16 ones-matmul (ones=4.0 folds the attT scale correction).
  attT stored as fp8 hi+lo; out[t, d] = attT.T @ w_o_rows via 3-term DR.
  Host divides the gathered partial sums by the net 2048x scale.

Scales: w_qkv and w_o are pre-scaled x64 into fp8 (e4m3 = IEEE variant,
  max finite 240); hidden stays x1. qkv = 64x, scores = 4096 s (folded
  into exp scale), probs = p/4 (exp bias), att = 16*Sum p v, colsum = p/4
  summed, ones=4.0 => attT = 16*attended, out = 1024*true. Host divides.
"""

import numpy as np
import ml_dtypes
from collections import deque
from contextlib import ExitStack

import concourse.bass as bass
import concourse.tile as tile
import concourse.mybir as mybir
from concourse.bass import ds, ts
from concourse.bass_utils import run_bass_kernel_spmd
from concourse.masks import make_identity

BF16 = mybir.dt.bfloat16
F16 = mybir.dt.float16
F32 = mybir.dt.float32
FP8 = mybir.dt.float8e4
DR = mybir.MatmulPerfMode.DoubleRow
P = 128
S = 2048          # sequence length
D = 2048          # d_model
NF = 6            # feature chunks of 128: 4 q heads, 1 k, 1 v
NPR = 8           # DR contraction pairs over d_model (2048 = 8*256)
NQT = S // 512    # 4 query tiles of 512
NTT = S // 512    # 4 token tiles of 512
WSCALE = 64.0     # fp8 pre-scale on w_qkv and w_o (e4m3 max is 240)
ISQ = 1.0 / np.sqrt(128.0)
EXP_SCALE = ISQ / (WSCALE * WSCALE)   # scores PSUM holds 4096*s
EXP_BIAS = float(-2.0 * np.log(2.0))  # probs = p/4 (fp16 colsum headroom)
OUT_DIV = 1024.0  # 16 (attT) * 64 (w_o)
MULT = mybir.AluOpType.mult
ADD = mybir.AluOpType.add

_CACHE = {}


def _legalize_waits(nc):
    """Split multi-wait sync_info into preceding single-wait engine NOPs.

    The walrus codegen in this container accepts at most ONE sync wait per
    TPB instruction ("Too many sync wait commands"), but the Tile scheduler
    freely emits several. An engine executes its queue in order, so hoisting
    the extra waits onto NoOps right before the instruction is equivalent.
    """
    n = 0
    for f in nc.m.functions:
        for blk in f.blocks:
            out = []
            changed = False
            for inst in blk.instructions:
                si = inst.sync_info
                if (si is not None and si.on_wait and len(si.on_wait) > 1
                        and str(inst.engine) != "EngineType.Unassigned"):
                    waits = list(si.on_wait)
                    for w in waits[:-1]:
                        out.append(mybir.InstNoOp(
                            name=f"I-wf{n}", engine=inst.engine, ins=[],
                            outs=[],
                            sync_info=mybir.SyncInfo(on_wait=[w],
                                                     on_update=[])))
                        n += 1
                    si.on_wait = [waits[-1]]
                    changed = True
                out.append(inst)
            if changed:
                blk.instructions = out
    return n


def _build(legalize=True, debug=False):
    key = ("nc" if legalize else "nc_raw") + ("_dbg" if debug else "")
    if key in _CACHE:
        return _CACHE[key]
    nc = bass.Bass("TRN2", target_bir_lowering=False, debug=False)

    hh_d = nc.dram_tensor("h_hi", [P, NPR, 2, S], FP8, kind="ExternalInput").ap()
    hl_d = nc.dram_tensor("h_lo", [P, NPR, 2, S], FP8, kind="ExternalInput").ap()
    wh_d = nc.dram_tensor("w_hi", [P, NF, NPR, 2, P], FP8, kind="ExternalInput").ap()
    wl_d = nc.dram_tensor("w_lo", [P, NF, NPR, 2, P], FP8, kind="ExternalInput").ap()
    oh_d = nc.dram_tensor("wo_hi", [P, 2, 2, D], FP8, kind="ExternalInput").ap()
    ol_d = nc.dram_tensor("wo_lo", [P, 2, 2, D], FP8, kind="ExternalInput").ap()
    cw_d = nc.dram_tensor("conv_w", [P, NF * 4], F32, kind="ExternalInput").ap()
    out_d = nc.dram_tensor("out", [S, D], F32, kind="ExternalOutput").ap()
    if debug:
        dbg_qkvb = nc.dram_tensor("dbg_qkvb", [P, NF, S], BF16,
                                  kind="ExternalOutput").ap()
        dbg_atth = nc.dram_tensor("dbg_atth", [P, 4, S], FP8,
                                  kind="ExternalOutput").ap()
        dbg_qkvf8 = nc.dram_tensor("dbg_qkvf8", [P, NF, S + 3], BF16,
                                   kind="ExternalOutput").ap()

    out_v = out_d.rearrange("(po pi) d -> pi po d", pi=P)      # [128,16,2048]

    with tile.TileContext(nc) as tc, ExitStack() as ctx:
        const = ctx.enter_context(tc.tile_pool(name="const", bufs=1))
        p_ht = ctx.enter_context(tc.tile_pool(name="ht", bufs=2))
        p_work = ctx.enter_context(tc.tile_pool(name="work", bufs=2))
        p_probs = ctx.enter_context(tc.tile_pool(name="probs", bufs=3))
        p_out = ctx.enter_context(tc.tile_pool(name="outp", bufs=6))
        ps2 = ctx.enter_context(tc.tile_pool(name="ps2", bufs=2, space="PSUM"))
        ps_s = ctx.enter_context(tc.tile_pool(name="ps_s", bufs=3, space="PSUM"))
        ps3 = ctx.enter_context(tc.tile_pool(name="ps3", bufs=2, space="PSUM"))
        ps1 = ctx.enter_context(tc.tile_pool(name="ps1", bufs=1, space="PSUM"))

        # --- constants / persistent tensors ---
        ident = const.tile([P, P], BF16, tag="ident")
        make_identity(nc, ident)
        cw0 = const.tile([P, NF * 4], F32, tag="cw0")
        nc.sync.dma_start(cw0, cw_d)
        # conv ops read cw via a DVE copy so their DMA wait lands here, not
        # on the (wait-slot-limited) Pool scalar_tensor_tensor instructions
        cw = const.tile([P, NF * 4], F32, tag="cw")
        nc.vector.tensor_copy(cw, cw0)
        wq_hi = const.tile([P, NF, NPR, 2, P], FP8, tag="wqh")
        wq_lo = const.tile([P, NF, NPR, 2, P], FP8, tag="wql")
        wo_hi = const.tile([P, 2, 2, D], FP8, tag="woh")
        wo_lo = const.tile([P, 2, 2, D], FP8, tag="wol")
        # raw (pre-conv) qkv.T in bf16 (64x scale), 3 leading zero columns so
        # the causal conv taps can read t-3..t-1 without edge cases
        qkvf = const.tile([P, NF, S + 3], BF16, tag="qkvf")
        nc.gpsimd.memset(qkvf[:, :, 0:3], 0.0)
        qkvb = const.tile([P, NF, S], BF16, tag="qkvb")    # conv'd qkv.T
        vnat = const.tile([P, 16, P], BF16, tag="vnat")    # v in [token, dh]
        atth = const.tile([P, 4, S], FP8, tag="atth")      # attT hi per head
        attl = const.tile([P, 4, S], FP8, tag="attl")      # attT lo per head
        ones2 = const.tile([P, P], F16, tag="ones2")
        nc.vector.memset(ones2, 4.0)
        ebias = const.tile([P, 1], F32, tag="ebias")
        nc.vector.memset(ebias, EXP_BIAS)

        def o_proj_chunk(qt, t4, final=False):
            # output projection for one token-128-tile of q-tile qt
            tt16 = qt * 4 + t4
            for dt in range(4):
                op = ps2.tile([P, 512], F32, tag="proj")
                k = 0
                for lhs, rhs_w in ((atth, wo_hi), (attl, wo_hi), (atth, wo_lo)):
                    for pr_ in range(2):
                        nc.tensor.matmul(
                            op,
                            lhsT=lhs[:, ds(2 * pr_, 2), ds(tt16 * P, P)],
                            rhs=rhs_w[:, pr_, :, ds(dt * 512, 512)],
                            start=(k == 0), stop=(k == 5), perf_mode=DR)
                        k += 1
                ob = p_out.tile([P, 512], F32, tag="ob")
                if dt == 3 and not final:
                    nc.scalar.copy(ob, op)
                else:
                    nc.vector.tensor_copy(ob, op)
                nc.sync.dma_start(out_v[:, tt16, ds(dt * 512, 512)], ob)

        def attn_B(qt):
            # attention for q-tile qt (needs token tiles <= qt). The four
            # heads are software-pipelined into ONE flat (h, kt) sequence:
            # scores of head h+1 are emitted while head h's attended matmuls
            # drain, so the in-order PE queue never stalls on the
            # exp (ACT) -> mask (Pool) producer chain. The previous q-tile's
            # output projection is interleaved as additional PE filler.
            nk = 4 * (qt + 1)
            LAG = 6
            state = {}  # h -> (att, colsum)
            fin = {}    # h -> (att, colsum) awaiting denominator finalize
            fin_q = deque()  # [h, consumes-since-ready]
            pend = deque()
            pr_quad = None

            def consume():
                ch, ppr, px0, pkt = pend.popleft()
                att, colsum = state[ch] if ch in state else fin[ch]
                nc.tensor.matmul(
                    att[:, px0:512], lhsT=vnat[:, pkt, :],
                    rhs=ppr[:, px0:512],
                    start=(pkt == 0), stop=(pkt == nk - 1))
                # softmax denominator: accumulate exp'd probs on DVE
                # (partition dim reduced by ONE ones-matmul at the end)
                if pkt == 0:
                    nc.vector.tensor_copy(colsum, ppr)
                else:
                    nc.vector.tensor_add(
                        colsum[:, px0:512], colsum[:, px0:512],
                        ppr[:, px0:512])
                if pkt == nk - 1:
                    fin[ch] = state.pop(ch)
                    fin_q.append([ch, 0])

            def finalize(ch):
                att, colsum = fin[ch]
                smp = ps1.tile([P, 512], F32, tag="small")
                nc.tensor.matmul(smp, lhsT=ones2, rhs=colsum,
                                 start=True, stop=True)
                rec = p_work.tile([P, 512], F32, tag="rec")
                nc.vector.reciprocal(rec, smp)
                t16 = p_work.tile([P, 512], F16, tag="t16")
                if qt == NQT - 1 and ch == 3:
                    # last head before the final output projection: emit the
                    # normalization in 128-col pieces so the first final
                    # o_proj chunks can start before the whole head is done
                    for pc in range(4):
                        c = ds(pc * P, P)
                        nc.vector.tensor_mul(t16[:, c], att[:, c], rec[:, c])
                        nc.gpsimd.tensor_copy(
                            atth[:, ch, ds(qt * 512 + pc * P, P)], t16[:, c])
                        nc.gpsimd.tensor_sub(
                            attl[:, ch, ds(qt * 512 + pc * P, P)], t16[:, c],
                            atth[:, ch, ds(qt * 512 + pc * P, P)])
                else:
                    nc.vector.tensor_mul(t16, att, rec)
                    nc.gpsimd.tensor_copy(atth[:, ch, ts(qt, 512)], t16)
                    nc.gpsimd.tensor_sub(attl[:, ch, ts(qt, 512)], t16,
                                         atth[:, ch, ts(qt, 512)])
                del fin[ch]

            for h in range(4):
                state[h] = (ps3.tile([P, 512], F32, tag="att", name="att"),
                            p_work.tile([P, 512], F16, tag="colsum",
                                        name="colsum"))
                for kt in range(nk):
                    if kt == min(4, nk - 2) and qt > 0:
                        # previous q-tile's output projection emitted mid-head
                        o_proj_chunk(qt - 1, h)
                    j = kt - 4 * qt
                    x0 = j * P if j >= 0 else 0
                    F = 512 - x0
                    sp = ps_s.tile([P, 512], F32, tag="s")
                    nc.tensor.matmul(
                        sp[:, x0:512],
                        lhsT=qkvb[:, 4, ds(kt * P, P)],
                        rhs=qkvb[:, h, ds(qt * 512 + x0, F)],
                        start=True, stop=True,
                    )
                    if kt % 4 == 0:
                        pr_quad = p_probs.tile([P, 4, 512], BF16, tag="probs")
                    pr = pr_quad[:, kt % 4, :]
                    nc.scalar.activation(
                        pr[:, x0:512], sp[:, x0:512],
                        mybir.ActivationFunctionType.Exp,
                        scale=EXP_SCALE, bias=ebias)
                    if j >= 0:
                        # zero the k>q half of the diagonal tile in place
                        # (local col c vs partition p: keep iff c >= p)
                        nc.gpsimd.affine_select(
                            out=pr[:, x0:512], in_=pr[:, x0:512],
                            pattern=[[1, F]], base=0,
                            channel_multiplier=-1,
                            compare_op=mybir.AluOpType.is_ge, fill=0.0)
                    pend.append((h, pr, x0, kt))
                    if len(pend) > LAG:
                        consume()
                        for e in fin_q:
                            e[1] += 1
                        if fin_q and fin_q[0][1] >= 2:
                            finalize(fin_q.popleft()[0])
            while pend:
                consume()
            while fin_q:
                finalize(fin_q.popleft()[0])

        # ------- Fused phases: per token tile: projection+conv, then the
        # attention q-tile that just became computable, then the (pipelined)
        # output projection of the previous q-tile.
        for tt in range(NTT):
            ht_hi = p_ht.tile([P, NPR, 2, 512], FP8, tag="hth")
            ht_lo = p_ht.tile([P, NPR, 2, 512], FP8, tag="htl")
            if tt == 0:
                # weights stream on the ACT hwdge queue (idle at startup), in
                # the same per-fc order the projection chains consume them;
                # hidden-state chunks stream in parallel on the SP queue
                for fc in (4, 5, 0, 1, 2, 3):
                    nc.scalar.dma_start(wq_hi[:, fc], wh_d[:, fc])
                    nc.scalar.dma_start(wq_lo[:, fc], wl_d[:, fc])
            for k2 in range(4):
                nc.sync.dma_start(ht_hi[:, ds(k2 * 2, 2)],
                                  hh_d[:, ds(k2 * 2, 2), :, ts(tt, 512)])
                nc.sync.dma_start(ht_lo[:, ds(k2 * 2, 2)],
                                  hl_d[:, ds(k2 * 2, 2), :, ts(tt, 512)])
            t0 = tt * 512

            for fc in (4, 5, 0, 1, 2, 3):
                pp = ps2.tile([P, 512], F32, tag="proj", name="pp")
                k = 0
                for lhs, rhs_h in ((wq_hi, ht_hi), (wq_hi, ht_lo),
                                   (wq_lo, ht_hi)):
                    for pr_ in range(NPR):
                        nc.tensor.matmul(
                            pp, lhsT=lhs[:, fc, pr_],
                            rhs=rhs_h[:, pr_],
                            start=(k == 0), stop=(k == 3 * NPR - 1),
                            perf_mode=DR)
                        k += 1
                # pre-conv x (64x) -> bf16 for the Pool conv taps
                nc.scalar.copy(qkvf[:, fc, ds(3 + t0, 512)], pp)
                # conv taps: out[t] = x[t] + sum_k x[t+k-3]*w[k]
                tmp = p_work.tile([P, 512], BF16, tag="ctmp", name="ctmp")
                nc.vector.scalar_tensor_tensor(
                    tmp, qkvf[:, fc, ds(t0 + 0, 512)],
                    cw[:, fc * 4 + 0: fc * 4 + 1],
                    qkvf[:, fc, ds(t0 + 3, 512)], op0=MULT, op1=ADD)
                nc.vector.scalar_tensor_tensor(
                    tmp, qkvf[:, fc, ds(t0 + 1, 512)],
                    cw[:, fc * 4 + 1: fc * 4 + 2], tmp, op0=MULT, op1=ADD)
                nc.vector.scalar_tensor_tensor(
                    tmp, qkvf[:, fc, ds(t0 + 2, 512)],
                    cw[:, fc * 4 + 2: fc * 4 + 3], tmp, op0=MULT, op1=ADD)
                nc.vector.scalar_tensor_tensor(
                    qkvb[:, fc, ts(tt, 512)], qkvf[:, fc, ds(t0 + 3, 512)],
                    cw[:, fc * 4 + 3: fc * 4 + 4], tmp, op0=MULT, op1=ADD)
            # v (fc=5) of this token tile -> natural [token, dh] layout
            for j in range(4):
                kt_i = tt * 4 + j
                trp = ps1.tile([P, 512], BF16, tag="small")
                nc.tensor.transpose(trp[:, 0:P], qkvb[:, 5, ds(kt_i * P, P)],
                                    ident)
                nc.vector.tensor_copy(vnat[:, kt_i, :], trp[:, 0:P])
            if tt == 0:
                # w_o load deferred past the critical head DMAs
                nc.sync.dma_start(wo_hi, oh_d)
                nc.sync.dma_start(wo_lo, ol_d)
            attn_B(tt)
        for t4 in range(4):
            o_proj_chunk(NQT - 1, t4, final=True)
        if debug:
            nc.sync.dma_start(dbg_qkvb, qkvb)
            nc.sync.dma_start(dbg_atth, atth)
            nc.sync.dma_start(dbg_qkvf8, qkvf)

    if legalize:
        _legalize_waits(nc)
    _CACHE[key] = nc
    return nc


def _prep_inputs(hidden_states, w_q, w_k, w_v, w_o, conv_w):
    """Build the 8 per-core input maps (host-side shard + fp8 hi/lo split)."""
    f8 = ml_dtypes.float8_e4m3

    def pairs(x, free):  # [2048, free] -> [128, 8, 2, free]
        return np.ascontiguousarray(
            x.reshape(NPR, 2, P, free).transpose(2, 0, 1, 3))

    def wpairs(x):  # [2048, 768] -> [128, 6, 8, 2, 128]
        return np.ascontiguousarray(
            x.reshape(NPR, 2, P, NF, P).transpose(2, 3, 0, 1, 4))

    def split8(x):
        hi = x.astype(f8)
        lo = (x - hi.astype(np.float32)).astype(f8)
        return hi, lo

    # hidden split is shared by the 4 cores of a batch
    h_pairs = []
    for b in range(2):
        hT = np.ascontiguousarray(hidden_states[b].T)
        hi, lo = split8(hT)
        h_pairs.append((pairs(hi, S), pairs(lo, S)))

    in_maps = []
    for c in range(8):
        b, g = c // 4, c % 4
        wqkv = np.concatenate(
            [w_q[:, g * 512:(g + 1) * 512],
             w_k[:, g * 128:(g + 1) * 128],
             w_v[:, g * 128:(g + 1) * 128]], axis=1) * WSCALE
        w_hi, w_lo = split8(wqkv)
        wo = np.ascontiguousarray(w_o[g * 512:(g + 1) * 512, :]) * WSCALE
        wo_hi, wo_lo = split8(wo)
        wo_hi = np.ascontiguousarray(
            wo_hi.reshape(2, 2, P, D).transpose(2, 0, 1, 3))
        wo_lo = np.ascontiguousarray(
            wo_lo.reshape(2, 2, P, D).transpose(2, 0, 1, 3))
        cw = np.concatenate(
            [conv_w[g * 512:(g + 1) * 512],
             conv_w[2048 + g * 128: 2048 + (g + 1) * 128],
             conv_w[2560 + g * 128: 2560 + (g + 1) * 128]], axis=0)  # [768,4]
        cwp = np.ascontiguousarray(
            cw.reshape(NF, P, 4).transpose(1, 0, 2).reshape(P, NF * 4)
        ).astype(np.float32)
        in_maps.append({
            "h_hi": h_pairs[b][0], "h_lo": h_pairs[b][1],
            "w_hi": wpairs(w_hi), "w_lo": wpairs(w_lo),
            "wo_hi": wo_hi, "wo_lo": wo_lo,
            "conv_w": cwp,
        })
    return in_maps


def kernel(hidden_states, w_q, w_k, w_v, w_o, conv_w, _trace=False):
    nc = _build()
    in_maps = _prep_inputs(
        np.asarray(hidden_states, dtype=np.float32),
        np.asarray(w_q, dtype=np.float32),
        np.asarray(w_k, dtype=np.float32),
        np.asarray(w_v, dtype=np.float32),
        np.asarray(w_o, dtype=np.float32),
        np.asarray(conv_w, dtype=np.float32),
    )
    res = run_bass_kernel_spmd(nc, in_maps, core_ids=list(range(8)),
                               trace=_trace)
    outs = [r["out"] for r in res.results]
    full = np.empty((2, S, D), dtype=np.float32)
    for b in range(2):
        acc = outs[4 * b] + outs[4 * b + 1] + outs[4 * b + 2] + outs[4 * b + 3]
        full[b] = acc * (1.0 / OUT_DIV)
    if _trace:
        kernel.last_results = res
    return full


# revision 36
# speedup vs baseline: 1.1606x; 1.0227x over previous
"""CanonCausalMultiheadAttn Trainium2 kernel (fp8 DoubleRow version).

Sharding: 8 cores = 2 (batch) x 4 (kv-head groups). Core c handles batch
c//4 and kv-group g=c%4 (q heads 4g..4g+3, kv head g). w_q/w_k/w_v are
column-sharded by head group, w_o row-sharded; each core emits a partial
[S, D] output which the host sums over the 4 groups of its batch.

Per-core dataflow (transposed [feature, token] layout; v transposed on PE):
  qkvT[f, t] = w_qkv[:, f].T @ hT[:, t]   -- fp8e4m3 DoubleRow matmuls with
      3-term hi/lo compensation (w_hi.h_hi + w_lo.h_hi + w_hi.h_lo), which
      matches bf16 accuracy at 0.75x the PE time (DR = 0.5 cy/row, 256-deep
      contraction per instruction).
  conv: depthwise causal taps in bf16 on DVE (per-partition-scalar
      mult-add along the free axis); weights stay near-exact.
  scores.T[k, q] = kT.T @ qT (bf16) -> exp on ACT (scale folds the fp8
      pre-scales; bias -2ln2 keeps fp16 column sums in range)
  causal: k-tiles with k0 <= q_end only; diagonal tiles masked in-place
      by an affine_select on the (otherwise idle) Pool engine.
  attT[dh, q] += v_nat[k,:].T @ probsT  (bf16)
  colsum via DVE adds in fp16 (2x DVE mode), partition-reduced by one
      fp16 ones-matmul (ones=4.0 folds the attT scale correction).
  attT stored as fp8 hi+lo; out[t, d] = attT.T @ w_o_rows via 3-term DR.
  Host divides the gathered partial sums by the net 2048x scale.

Scales: w_qkv and w_o are pre-scaled x64 into fp8 (e4m3 = IEEE variant,
  max finite 240); hidden stays x1. qkv = 64x, scores = 4096 s (folded
  into exp scale), probs = p/4 (exp bias), att = 16*Sum p v, colsum = p/4
  summed, ones=4.0 => attT = 16*attended, out = 1024*true. Host divides.
"""

import numpy as np
import ml_dtypes
from collections import deque
from contextlib import ExitStack

import concourse.bass as bass
import concourse.tile as tile
import concourse.mybir as mybir
from concourse.bass import ds, ts
from concourse.bass_utils import run_bass_kernel_spmd
from concourse.masks import make_identity

BF16 = mybir.dt.bfloat16
F16 = mybir.dt.float16
F32 = mybir.dt.float32
FP8 = mybir.dt.float8e4
DR = mybir.MatmulPerfMode.DoubleRow
P = 128
S = 2048          # sequence length
D = 2048          # d_model
NF = 6            # feature chunks of 128: 4 q heads, 1 k, 1 v
NPR = 8           # DR contraction pairs over d_model (2048 = 8*256)
NQT = S // 512    # 4 query tiles of 512
NTT = S // 512    # 4 token tiles of 512
WSCALE = 64.0     # fp8 pre-scale on w_qkv and w_o (e4m3 max is 240)
ISQ = 1.0 / np.sqrt(128.0)
EXP_SCALE = ISQ / (WSCALE * WSCALE)   # scores PSUM holds 4096*s
EXP_BIAS = float(-2.0 * np.log(2.0))  # probs = p/4 (fp16 colsum headroom)
OUT_DIV = 1024.0  # 16 (attT) * 64 (w_o)
MULT = mybir.AluOpType.mult
ADD = mybir.AluOpType.add

_CACHE = {}


def _legalize_waits(nc):
    """Split multi-wait sync_info into preceding single-wait engine NOPs.

    The walrus codegen in this container accepts at most ONE sync wait per
    TPB instruction ("Too many sync wait commands"), but the Tile scheduler
    freely emits several. An engine executes its queue in order, so hoisting
    the extra waits onto NoOps right before the instruction is equivalent.
    """
    n = 0
    for f in nc.m.functions:
        for blk in f.blocks:
            out = []
            changed = False
            for inst in blk.instructions:
                si = inst.sync_info
                if (si is not None and si.on_wait and len(si.on_wait) > 1
                        and str(inst.engine) != "EngineType.Unassigned"):
                    waits = list(si.on_wait)
                    for w in waits[:-1]:
                        out.append(mybir.InstNoOp(
                            name=f"I-wf{n}", engine=inst.engine, ins=[],
                            outs=[],
                            sync_info=mybir.SyncInfo(on_wait=[w],
                                                     on_update=[])))
                        n += 1
                    si.on_wait = [waits[-1]]
                    changed = True
                out.append(inst)
            if changed:
                blk.instructions = out
    return n


def _build(legalize=True, debug=False):
    key = ("nc" if legalize else "nc_raw") + ("_dbg" if debug else "")
    if key in _CACHE:
        return _CACHE[key]
    nc = bass.Bass("TRN2", target_bir_lowering=False, debug=False)

    hh_d = nc.dram_tensor("h_hi", [P, NPR, 2, S], FP8, kind="ExternalInput").ap()
    hl_d = nc.dram_tensor("h_lo", [P, NPR, 2, S], FP8, kind="ExternalInput").ap()
    wh_d = nc.dram_tensor("w_hi", [P, NF, NPR, 2, P], FP8, kind="ExternalInput").ap()
    wl_d = nc.dram_tensor("w_lo", [P, NF, NPR, 2, P], FP8, kind="ExternalInput").ap()
    oh_d = nc.dram_tensor("wo_hi", [P, 2, 2, D], FP8, kind="ExternalInput").ap()
    ol_d = nc.dram_tensor("wo_lo", [P, 2, 2, D], FP8, kind="ExternalInput").ap()
    cw_d = nc.dram_tensor("conv_w", [P, NF * 4], F32, kind="ExternalInput").ap()
    out_d = nc.dram_tensor("out", [S, D], F32, kind="ExternalOutput").ap()
    if debug:
        dbg_qkvb = nc.dram_tensor("dbg_qkvb", [P, NF, S], BF16,
                                  kind="ExternalOutput").ap()
        dbg_atth = nc.dram_tensor("dbg_atth", [P, 4, S], FP8,
                                  kind="ExternalOutput").ap()
        dbg_qkvf8 = nc.dram_tensor("dbg_qkvf8", [P, NF, S + 3], BF16,
                                   kind="ExternalOutput").ap()

    out_v = out_d.rearrange("(po pi) d -> pi po d", pi=P)      # [128,16,2048]

    with tile.TileContext(nc) as tc, ExitStack() as ctx:
        const = ctx.enter_context(tc.tile_pool(name="const", bufs=1))
        p_ht = ctx.enter_context(tc.tile_pool(name="ht", bufs=2))
        p_work = ctx.enter_context(tc.tile_pool(name="work", bufs=2))
        p_probs = ctx.enter_context(tc.tile_pool(name="probs", bufs=4))
        p_out = ctx.enter_context(tc.tile_pool(name="outp", bufs=6))
        ps2 = ctx.enter_context(tc.tile_pool(name="ps2", bufs=2, space="PSUM"))
        ps_s = ctx.enter_context(tc.tile_pool(name="ps_s", bufs=3, space="PSUM"))
        ps3 = ctx.enter_context(tc.tile_pool(name="ps3", bufs=2, space="PSUM"))
        ps1 = ctx.enter_context(tc.tile_pool(name="ps1", bufs=1, space="PSUM"))

        # --- constants / persistent tensors ---
        ident = const.tile([P, P], BF16, tag="ident")
        make_identity(nc, ident)
        cw0 = const.tile([P, NF * 4], F32, tag="cw0")
        nc.sync.dma_start(cw0, cw_d)
        # conv ops read cw via a DVE copy so their DMA wait lands here, not
        # on the (wait-slot-limited) Pool scalar_tensor_tensor instructions
        cw = const.tile([P, NF * 4], F32, tag="cw")
        nc.vector.tensor_copy(cw, cw0)
        wq_hi = const.tile([P, NF, NPR, 2, P], FP8, tag="wqh")
        wq_lo = const.tile([P, NF, NPR, 2, P], FP8, tag="wql")
        wo_hi = const.tile([P, 2, 2, D], FP8, tag="woh")
        wo_lo = const.tile([P, 2, 2, D], FP8, tag="wol")
        # raw (pre-conv) qkv.T in bf16 (64x scale), 3 leading zero columns so
        # the causal conv taps can read t-3..t-1 without edge cases
        qkvf = const.tile([P, NF, S + 3], BF16, tag="qkvf")
        nc.gpsimd.memset(qkvf[:, :, 0:3], 0.0)
        qkvb = const.tile([P, NF, S], BF16, tag="qkvb")    # conv'd qkv.T
        vnat = const.tile([P, 16, P], BF16, tag="vnat")    # v in [token, dh]
        atth = const.tile([P, 4, S], FP8, tag="atth")      # attT hi per head
        attl = const.tile([P, 4, S], FP8, tag="attl")      # attT lo per head
        ones2 = const.tile([P, P], F16, tag="ones2")
        nc.vector.memset(ones2, 4.0)
        ebias = const.tile([P, 1], F32, tag="ebias")
        nc.vector.memset(ebias, EXP_BIAS)

        def o_proj_chunk(qt, t4, final=False):
            # output projection for one token-128-tile of q-tile qt
            tt16 = qt * 4 + t4
            for dt in range(4):
                op = ps2.tile([P, 512], F32, tag="proj")
                k = 0
                for lhs, rhs_w in ((atth, wo_hi), (attl, wo_hi), (atth, wo_lo)):
                    for pr_ in range(2):
                        nc.tensor.matmul(
                            op,
                            lhsT=lhs[:, ds(2 * pr_, 2), ds(tt16 * P, P)],
                            rhs=rhs_w[:, pr_, :, ds(dt * 512, 512)],
                            start=(k == 0), stop=(k == 5), perf_mode=DR)
                        k += 1
                ob = p_out.tile([P, 512], F32, tag="ob")
                if dt == 3 or final:
                    nc.scalar.copy(ob, op)
                else:
                    nc.vector.tensor_copy(ob, op)
                nc.sync.dma_start(out_v[:, tt16, ds(dt * 512, 512)], ob)

        def attn_B(qt):
            # attention for q-tile qt (needs token tiles <= qt). The four
            # heads are software-pipelined into ONE flat (h, kt) sequence:
            # scores of head h+1 are emitted while head h's attended matmuls
            # drain, so the in-order PE queue never stalls on the
            # exp (ACT) -> mask (Pool) producer chain. The previous q-tile's
            # output projection is interleaved as additional PE filler.
            nk = 4 * (qt + 1)
            LAG = 10
            state = {}  # h -> (att, colsum)
            fin = {}    # h -> (att, colsum) awaiting denominator finalize
            fin_q = deque()  # [h, consumes-since-ready]
            pend = deque()
            pr_quad = None

            def consume():
                ch, ppr, px0, pkt = pend.popleft()
                att, colsum = state[ch] if ch in state else fin[ch]
                nc.tensor.matmul(
                    att[:, px0:512], lhsT=vnat[:, pkt, :],
                    rhs=ppr[:, px0:512],
                    start=(pkt == 0), stop=(pkt == nk - 1))
                # softmax denominator: accumulate exp'd probs on DVE
                # (partition dim reduced by ONE ones-matmul at the end)
                if pkt == 0:
                    nc.vector.tensor_copy(colsum, ppr)
                else:
                    nc.vector.tensor_add(
                        colsum[:, px0:512], colsum[:, px0:512],
                        ppr[:, px0:512])
                if pkt == nk - 1:
                    fin[ch] = state.pop(ch)
                    fin_q.append([ch, 0])

            def finalize(ch):
                att, colsum = fin[ch]
                smp = ps1.tile([P, 512], F32, tag="small")
                nc.tensor.matmul(smp, lhsT=ones2, rhs=colsum,
                                 start=True, stop=True)
                rec = p_work.tile([P, 512], F32, tag="rec")
                nc.vector.reciprocal(rec, smp)
                t16 = p_work.tile([P, 512], F16, tag="t16")
                if qt == NQT - 1 and ch == 3:
                    # last head before the final output projection: emit the
                    # normalization in 128-col pieces on DVE so the first
                    # final o_proj chunks start before the whole head is done
                    for pc in range(4):
                        c = ds(pc * P, P)
                        nc.vector.tensor_mul(t16[:, c], att[:, c], rec[:, c])
                        nc.vector.tensor_copy(
                            atth[:, ch, ds(qt * 512 + pc * P, P)], t16[:, c])
                        nc.vector.tensor_sub(
                            attl[:, ch, ds(qt * 512 + pc * P, P)], t16[:, c],
                            atth[:, ch, ds(qt * 512 + pc * P, P)])
                else:
                    nc.vector.tensor_mul(t16, att, rec)
                    nc.gpsimd.tensor_copy(atth[:, ch, ts(qt, 512)], t16)
                    nc.gpsimd.tensor_sub(attl[:, ch, ts(qt, 512)], t16,
                                         atth[:, ch, ts(qt, 512)])
                del fin[ch]

            for h in range(4):
                state[h] = (ps3.tile([P, 512], F32, tag="att", name="att"),
                            p_work.tile([P, 512], F16, tag="colsum",
                                        name="colsum"))
                for kt in range(nk):
                    if kt == min(4, nk - 2) and qt > 0:
                        # previous q-tile's output projection emitted mid-head
                        o_proj_chunk(qt - 1, h)
                    j = kt - 4 * qt
                    x0 = j * P if j >= 0 else 0
                    F = 512 - x0
                    sp = ps_s.tile([P, 512], F32, tag="s")
                    nc.tensor.matmul(
                        sp[:, x0:512],
                        lhsT=qkvb[:, 4, ds(kt * P, P)],
                        rhs=qkvb[:, h, ds(qt * 512 + x0, F)],
                        start=True, stop=True,
                    )
                    if kt % 4 == 0:
                        pr_quad = p_probs.tile([P, 4, 512], BF16, tag="probs")
                    pr = pr_quad[:, kt % 4, :]
                    nc.scalar.activation(
                        pr[:, x0:512], sp[:, x0:512],
                        mybir.ActivationFunctionType.Exp,
                        scale=EXP_SCALE, bias=ebias)
                    if j >= 0:
                        # zero the k>q half of the diagonal tile in place
                        # (local col c vs partition p: keep iff c >= p)
                        nc.gpsimd.affine_select(
                            out=pr[:, x0:512], in_=pr[:, x0:512],
                            pattern=[[1, F]], base=0,
                            channel_multiplier=-1,
                            compare_op=mybir.AluOpType.is_ge, fill=0.0)
                    pend.append((h, pr, x0, kt))
                    if len(pend) > LAG:
                        consume()
                        for e in fin_q:
                            e[1] += 1
                        if fin_q and fin_q[0][1] >= 2:
                            finalize(fin_q.popleft()[0])
            while pend:
                consume()
            while fin_q:
                finalize(fin_q.popleft()[0])

        # ------- Fused phases: per token tile: projection+conv, then the
        # attention q-tile that just became computable, then the (pipelined)
        # output projection of the previous q-tile.
        for tt in range(NTT):
            ht_hi = p_ht.tile([P, NPR, 2, 512], FP8, tag="hth")
            ht_lo = p_ht.tile([P, NPR, 2, 512], FP8, tag="htl")
            if tt == 0:
                # weights stream on the ACT hwdge queue (idle at startup), in
                # the same per-fc order the projection chains consume them;
                # hidden-state chunks stream in parallel on the SP queue
                for fc in (4, 5, 0, 1, 2, 3):
                    nc.scalar.dma_start(wq_hi[:, fc], wh_d[:, fc])
                    nc.scalar.dma_start(wq_lo[:, fc], wl_d[:, fc])
            for k2 in range(4):
                nc.sync.dma_start(ht_hi[:, ds(k2 * 2, 2)],
                                  hh_d[:, ds(k2 * 2, 2), :, ts(tt, 512)])
                nc.sync.dma_start(ht_lo[:, ds(k2 * 2, 2)],
                                  hl_d[:, ds(k2 * 2, 2), :, ts(tt, 512)])
            t0 = tt * 512

            def proj_group(pp, fc, g):
                lhs, rhs_h = ((wq_hi, ht_hi), (wq_hi, ht_lo),
                              (wq_lo, ht_hi))[g]
                for pr_ in range(NPR):
                    nc.tensor.matmul(
                        pp, lhsT=lhs[:, fc, pr_], rhs=rhs_h[:, pr_],
                        start=(g == 0 and pr_ == 0),
                        stop=(g == 2 and pr_ == NPR - 1), perf_mode=DR)

            def conv_fc(fc, pp):
                # pre-conv x (64x) -> bf16 for the DVE conv taps
                nc.scalar.copy(qkvf[:, fc, ds(3 + t0, 512)], pp)
                # conv taps: out[t] = x[t] + sum_k x[t+k-3]*w[k].
                # Products via tensor_scalar (4x DVE mode — the tensor-tensor
                # variant gets no fast mode), sums via bf16 tensor_tensor (2x)
                ca = p_work.tile([P, 512], BF16, tag="ctmpa", name="ca")
                cb = p_work.tile([P, 512], BF16, tag="ctmpb", name="cb")
                nc.vector.tensor_scalar(
                    ca, qkvf[:, fc, ds(t0 + 0, 512)],
                    cw[:, fc * 4 + 0: fc * 4 + 1], None, op0=MULT)
                nc.vector.tensor_scalar(
                    cb, qkvf[:, fc, ds(t0 + 1, 512)],
                    cw[:, fc * 4 + 1: fc * 4 + 2], None, op0=MULT)
                nc.vector.tensor_add(ca, ca, cb)
                nc.vector.tensor_scalar(
                    cb, qkvf[:, fc, ds(t0 + 2, 512)],
                    cw[:, fc * 4 + 2: fc * 4 + 3], None, op0=MULT)
                nc.vector.tensor_add(ca, ca, cb)
                nc.vector.tensor_scalar(
                    cb, qkvf[:, fc, ds(t0 + 3, 512)],
                    cw[:, fc * 4 + 3: fc * 4 + 4], None, op0=MULT)
                nc.vector.tensor_add(cb, cb, qkvf[:, fc, ds(t0 + 3, 512)])
                nc.vector.tensor_add(qkvb[:, fc, ts(tt, 512)], ca, cb)

            # pairwise-interleaved projection chains (2 PSUM banks): the
            # second chain's hi-groups execute while the first chain's
            # lo-operand DMAs are still landing
            for fa, fb in ((4, 5), (0, 1), (2, 3)):
                pa = ps2.tile([P, 512], F32, tag="proj", name="pa")
                pb = ps2.tile([P, 512], F32, tag="proj", name="pb")
                for g in range(3):
                    proj_group(pa, fa, g)
                    proj_group(pb, fb, g)
                conv_fc(fa, pa)
                conv_fc(fb, pb)
            # v (fc=5) of this token tile -> natural [token, dh] layout
            trp = ps1.tile([P, 512], BF16, tag="small")
            for j in range(4):
                nc.tensor.transpose(trp[:, ds(j * P, P)],
                                    qkvb[:, 5, ds((tt * 4 + j) * P, P)],
                                    ident)
            nc.vector.tensor_copy(vnat[:, ds(tt * 4, 4), :], trp)
            if tt == 0:
                # w_o load deferred past the critical head DMAs
                nc.sync.dma_start(wo_hi, oh_d)
                nc.sync.dma_start(wo_lo, ol_d)
            attn_B(tt)
        for t4 in range(4):
            o_proj_chunk(NQT - 1, t4, final=True)
        if debug:
            nc.sync.dma_start(dbg_qkvb, qkvb)
            nc.sync.dma_start(dbg_atth, atth)
            nc.sync.dma_start(dbg_qkvf8, qkvf)

    if legalize:
        _legalize_waits(nc)
    _CACHE[key] = nc
    return nc


def _prep_inputs(hidden_states, w_q, w_k, w_v, w_o, conv_w):
    """Build the 8 per-core input maps (host-side shard + fp8 hi/lo split)."""
    f8 = ml_dtypes.float8_e4m3

    def pairs(x, free):  # [2048, free] -> [128, 8, 2, free]
        return np.ascontiguousarray(
            x.reshape(NPR, 2, P, free).transpose(2, 0, 1, 3))

    def wpairs(x):  # [2048, 768] -> [128, 6, 8, 2, 128]
        return np.ascontiguousarray(
            x.reshape(NPR, 2, P, NF, P).transpose(2, 3, 0, 1, 4))

    def split8(x):
        hi = x.astype(f8)
        lo = (x - hi.astype(np.float32)).astype(f8)
        return hi, lo

    # hidden split is shared by the 4 cores of a batch
    h_pairs = []
    for b in range(2):
        hT = np.ascontiguousarray(hidden_states[b].T)
        hi, lo = split8(hT)
        h_pairs.append((pairs(hi, S), pairs(lo, S)))

    in_maps = []
    for c in range(8):
        b, g = c // 4, c % 4
        wqkv = np.concatenate(
            [w_q[:, g * 512:(g + 1) * 512],
             w_k[:, g * 128:(g + 1) * 128],
             w_v[:, g * 128:(g + 1) * 128]], axis=1) * WSCALE
        w_hi, w_lo = split8(wqkv)
        wo = np.ascontiguousarray(w_o[g * 512:(g + 1) * 512, :]) * WSCALE
        wo_hi, wo_lo = split8(wo)
        wo_hi = np.ascontiguousarray(
            wo_hi.reshape(2, 2, P, D).transpose(2, 0, 1, 3))
        wo_lo = np.ascontiguousarray(
            wo_lo.reshape(2, 2, P, D).transpose(2, 0, 1, 3))
        cw = np.concatenate(
            [conv_w[g * 512:(g + 1) * 512],
             conv_w[2048 + g * 128: 2048 + (g + 1) * 128],
             conv_w[2560 + g * 128: 2560 + (g + 1) * 128]], axis=0)  # [768,4]
        cwp = np.ascontiguousarray(
            cw.reshape(NF, P, 4).transpose(1, 0, 2).reshape(P, NF * 4)
        ).astype(np.float32)
        in_maps.append({
            "h_hi": h_pairs[b][0], "h_lo": h_pairs[b][1],
            "w_hi": wpairs(w_hi), "w_lo": wpairs(w_lo),
            "wo_hi": wo_hi, "wo_lo": wo_lo,
            "conv_w": cwp,
        })
    return in_maps


def kernel(hidden_states, w_q, w_k, w_v, w_o, conv_w, _trace=False):
    nc = _build()
    in_maps = _prep_inputs(
        np.asarray(hidden_states, dtype=np.float32),
        np.asarray(w_q, dtype=np.float32),
        np.asarray(w_k, dtype=np.float32),
        np.asarray(w_v, dtype=np.float32),
        np.asarray(w_o, dtype=np.float32),
        np.asarray(conv_w, dtype=np.float32),
    )
    res = run_bass_kernel_spmd(nc, in_maps, core_ids=list(range(8)),
                               trace=_trace)
    outs = [r["out"] for r in res.results]
    full = np.empty((2, S, D), dtype=np.float32)
    for b in range(2):
        acc = outs[4 * b] + outs[4 * b + 1] + outs[4 * b + 2] + outs[4 * b + 3]
        full[b] = acc * (1.0 / OUT_DIV)
    if _trace:
        kernel.last_results = res
    return full


# revision 50
# speedup vs baseline: 1.1706x; 1.0086x over previous
"""CanonCausalMultiheadAttn Trainium2 kernel (fp8 DoubleRow version).

Sharding: 8 cores = 2 (batch) x 4 (kv-head groups). Core c handles batch
c//4 and kv-group g=c%4 (q heads 4g..4g+3, kv head g). w_q/w_k/w_v are
column-sharded by head group, w_o row-sharded; each core emits a partial
[S, D] output which the host sums over the 4 groups of its batch.

The four heads of each q-tile are software-pipelined into one flat
(head, k-tile) sequence with a deep (LAG=10) probs queue, so the in-order
PE queue never stalls on the exp (ACT) -> causal-mask (Pool) producer
chain; the previous q-tile's output projection is interleaved as PE
filler. DMA dispatch cost is per-descriptor, so hidden states use a
tile-major host layout (fully contiguous lines, 2 large DMAs per tile)
and weights stream per-fc on the otherwise-idle ACT hwdge queue.

Per-core dataflow (transposed [feature, token] layout; v transposed on PE):
  qkvT[f, t] = w_qkv[:, f].T @ hT[:, t]   -- fp8e4m3 DoubleRow matmuls with
      3-term hi/lo compensation (w_hi.h_hi + w_lo.h_hi + w_hi.h_lo), which
      matches bf16 accuracy at 0.75x the PE time (DR = 0.5 cy/row, 256-deep
      contraction per instruction).
  conv: depthwise causal taps in bf16 on DVE: per-tap products via
      tensor_scalar (4x DVE mode), summed with bf16 tensor_tensor adds
      (2x mode); weights stay near-exact (fp8 conv weights measured 5e-3
      end-to-end error, so the taps are NOT quantized).
  scores.T[k, q] = kT.T @ qT (bf16) -> exp on ACT (scale folds the fp8
      pre-scales; bias -2ln2 keeps fp16 column sums in range)
  causal: k-tiles with k0 <= q_end only; diagonal tiles masked in-place
      by an affine_select on the (otherwise idle) Pool engine.
  attT[dh, q] += v_nat[k,:].T @ probsT  (bf16)
  colsum via DVE adds in fp16 (2x DVE mode), partition-reduced by one
      fp16 ones-matmul (ones=4.0 folds the attT scale correction).
  attT stored as fp8 hi+lo; out[t, d] = attT.T @ w_o_rows via 3-term DR.
  Host divides the gathered partial sums by the net 2048x scale.

Scales: w_qkv and w_o are pre-scaled x64 into fp8 (e4m3 = IEEE variant,
  max finite 240); hidden stays x1. qkv = 64x, scores = 4096 s (folded
  into exp scale), probs = p/4 (exp bias), att = 16*Sum p v, colsum = p/4
  summed, ones=4.0 => attT = 16*attended, out = 1024*true. Host divides.
"""

import numpy as np
import ml_dtypes
from collections import deque
from contextlib import ExitStack

import concourse.bass as bass
import concourse.tile as tile
import concourse.mybir as mybir
from concourse.bass import ds, ts
from concourse.bass_utils import run_bass_kernel_spmd
from concourse.masks import make_identity

BF16 = mybir.dt.bfloat16
F16 = mybir.dt.float16
F32 = mybir.dt.float32
FP8 = mybir.dt.float8e4
DR = mybir.MatmulPerfMode.DoubleRow
P = 128
S = 2048          # sequence length
D = 2048          # d_model
NF = 6            # feature chunks of 128: 4 q heads, 1 k, 1 v
NPR = 8           # DR contraction pairs over d_model (2048 = 8*256)
NQT = S // 512    # 4 query tiles of 512
NTT = S // 512    # 4 token tiles of 512
WSCALE = 64.0     # fp8 pre-scale on w_qkv and w_o (e4m3 max is 240)
ISQ = 1.0 / np.sqrt(128.0)
EXP_SCALE = ISQ / (WSCALE * WSCALE)   # scores PSUM holds 4096*s
EXP_BIAS = float(-2.0 * np.log(2.0))  # probs = p/4 (fp16 colsum headroom)
OUT_DIV = 1024.0  # 16 (attT) * 64 (w_o)
MULT = mybir.AluOpType.mult
ADD = mybir.AluOpType.add

_CACHE = {}


def _legalize_waits(nc):
    """Split multi-wait sync_info into preceding single-wait engine NOPs.

    The walrus codegen in this container accepts at most ONE sync wait per
    TPB instruction ("Too many sync wait commands"), but the Tile scheduler
    freely emits several. An engine executes its queue in order, so hoisting
    the extra waits onto NoOps right before the instruction is equivalent.
    """
    n = 0
    for f in nc.m.functions:
        for blk in f.blocks:
            out = []
            changed = False
            for inst in blk.instructions:
                si = inst.sync_info
                if (si is not None and si.on_wait and len(si.on_wait) > 1
                        and str(inst.engine) != "EngineType.Unassigned"):
                    waits = list(si.on_wait)
                    for w in waits[:-1]:
                        out.append(mybir.InstNoOp(
                            name=f"I-wf{n}", engine=inst.engine, ins=[],
                            outs=[],
                            sync_info=mybir.SyncInfo(on_wait=[w],
                                                     on_update=[])))
                        n += 1
                    si.on_wait = [waits[-1]]
                    changed = True
                out.append(inst)
            if changed:
                blk.instructions = out
    return n


def _build(legalize=True, debug=False):
    key = ("nc" if legalize else "nc_raw") + ("_dbg" if debug else "")
    if key in _CACHE:
        return _CACHE[key]
    nc = bass.Bass("TRN2", target_bir_lowering=False, debug=False)

    hh_d = nc.dram_tensor("h_hi", [P, NTT, NPR, 2, 512], FP8, kind="ExternalInput").ap()
    hl_d = nc.dram_tensor("h_lo", [P, NTT, NPR, 2, 512], FP8, kind="ExternalInput").ap()
    wh_d = nc.dram_tensor("w_hi", [P, NF, NPR, 2, P], FP8, kind="ExternalInput").ap()
    wl_d = nc.dram_tensor("w_lo", [P, NF, NPR, 2, P], FP8, kind="ExternalInput").ap()
    oh_d = nc.dram_tensor("wo_hi", [P, 2, 2, D], FP8, kind="ExternalInput").ap()
    ol_d = nc.dram_tensor("wo_lo", [P, 2, 2, D], FP8, kind="ExternalInput").ap()
    cw_d = nc.dram_tensor("conv_w", [P, NF * 4], F32, kind="ExternalInput").ap()
    out_d = nc.dram_tensor("out", [S, D], F32, kind="ExternalOutput").ap()
    if debug:
        dbg_qkvb = nc.dram_tensor("dbg_qkvb", [P, NF, S], BF16,
                                  kind="ExternalOutput").ap()
        dbg_atth = nc.dram_tensor("dbg_atth", [P, 4, S], FP8,
                                  kind="ExternalOutput").ap()
        dbg_qkvf8 = nc.dram_tensor("dbg_qkvf8", [P, NF, S + 3], BF16,
                                   kind="ExternalOutput").ap()

    out_v = out_d.rearrange("(po pi) d -> pi po d", pi=P)      # [128,16,2048]

    with tile.TileContext(nc) as tc, ExitStack() as ctx:
        const = ctx.enter_context(tc.tile_pool(name="const", bufs=1))
        p_ht = ctx.enter_context(tc.tile_pool(name="ht", bufs=2))
        p_work = ctx.enter_context(tc.tile_pool(name="work", bufs=2))
        p_probs = ctx.enter_context(tc.tile_pool(name="probs", bufs=4))
        p_out = ctx.enter_context(tc.tile_pool(name="outp", bufs=6))
        ps2 = ctx.enter_context(tc.tile_pool(name="ps2", bufs=2, space="PSUM"))
        ps_s = ctx.enter_context(tc.tile_pool(name="ps_s", bufs=3, space="PSUM"))
        ps3 = ctx.enter_context(tc.tile_pool(name="ps3", bufs=2, space="PSUM"))
        ps1 = ctx.enter_context(tc.tile_pool(name="ps1", bufs=1, space="PSUM"))

        # --- constants / persistent tensors ---
        ident = const.tile([P, P], BF16, tag="ident")
        make_identity(nc, ident)
        cw0 = const.tile([P, NF * 4], F32, tag="cw0")
        nc.scalar.dma_start(cw0, cw_d)
        # conv ops read cw via a DVE copy so their DMA wait lands here, not
        # on the (wait-slot-limited) Pool scalar_tensor_tensor instructions
        cw = const.tile([P, NF * 4], F32, tag="cw")
        nc.vector.tensor_copy(cw, cw0)
        wq_hi = const.tile([P, NF, NPR, 2, P], FP8, tag="wqh")
        wq_lo = const.tile([P, NF, NPR, 2, P], FP8, tag="wql")
        wo_hi = const.tile([P, 2, 2, D], FP8, tag="woh")
        wo_lo = const.tile([P, 2, 2, D], FP8, tag="wol")
        # raw (pre-conv) qkv.T in bf16 (64x scale), 3 leading zero columns so
        # the causal conv taps can read t-3..t-1 without edge cases
        qkvf = const.tile([P, NF, S + 3], BF16, tag="qkvf")
        nc.gpsimd.memset(qkvf[:, :, 0:3], 0.0)
        qkvb = const.tile([P, NF, S], BF16, tag="qkvb")    # conv'd qkv.T
        vnat = const.tile([P, 16, P], BF16, tag="vnat")    # v in [token, dh]
        atth = const.tile([P, 4, S], FP8, tag="atth")      # attT hi per head
        attl = const.tile([P, 4, S], FP8, tag="attl")      # attT lo per head
        ones2 = const.tile([P, P], F16, tag="ones2")
        nc.vector.memset(ones2, 4.0)
        ebias = const.tile([P, 1], F32, tag="ebias")
        nc.vector.memset(ebias, EXP_BIAS)

        def o_proj_chunk(qt, t4, final=False):
            # output projection for one token-128-tile of q-tile qt
            tt16 = qt * 4 + t4
            for dt in range(4):
                op = ps2.tile([P, 512], F32, tag="proj")
                k = 0
                for lhs, rhs_w in ((atth, wo_hi), (attl, wo_hi), (atth, wo_lo)):
                    for pr_ in range(2):
                        nc.tensor.matmul(
                            op,
                            lhsT=lhs[:, ds(2 * pr_, 2), ds(tt16 * P, P)],
                            rhs=rhs_w[:, pr_, :, ds(dt * 512, 512)],
                            start=(k == 0), stop=(k == 5), perf_mode=DR)
                        k += 1
                ob = p_out.tile([P, 512], F32, tag="ob")
                if dt == 3 or final:
                    nc.scalar.copy(ob, op)
                else:
                    nc.vector.tensor_copy(ob, op)
                nc.sync.dma_start(out_v[:, tt16, ds(dt * 512, 512)], ob)

        def attn_B(qt):
            # attention for q-tile qt (needs token tiles <= qt). The four
            # heads are software-pipelined into ONE flat (h, kt) sequence:
            # scores of head h+1 are emitted while head h's attended matmuls
            # drain, so the in-order PE queue never stalls on the
            # exp (ACT) -> mask (Pool) producer chain. The previous q-tile's
            # output projection is interleaved as additional PE filler.
            nk = 4 * (qt + 1)
            LAG = 10
            state = {}  # h -> (att, colsum)
            fin = {}    # h -> (att, colsum) awaiting denominator finalize
            fin_q = deque()  # [h, consumes-since-ready]
            pend = deque()
            pr_quad = None

            def consume():
                ch, ppr, px0, pkt = pend.popleft()
                att, colsum = state[ch] if ch in state else fin[ch]
                nc.tensor.matmul(
                    att[:, px0:512], lhsT=vnat[:, pkt, :],
                    rhs=ppr[:, px0:512],
                    start=(pkt == 0), stop=(pkt == nk - 1))
                # softmax denominator: accumulate exp'd probs on DVE
                # (partition dim reduced by ONE ones-matmul at the end)
                if pkt == 0:
                    nc.vector.tensor_copy(colsum, ppr)
                else:
                    nc.vector.tensor_add(
                        colsum[:, px0:512], colsum[:, px0:512],
                        ppr[:, px0:512])
                if pkt == nk - 1:
                    fin[ch] = state.pop(ch)
                    fin_q.append([ch, 0])

            def finalize(ch):
                att, colsum = fin[ch]
                smp = ps1.tile([P, 512], F32, tag="small")
                nc.tensor.matmul(smp, lhsT=ones2, rhs=colsum,
                                 start=True, stop=True)
                rec = p_work.tile([P, 512], F32, tag="rec")
                nc.vector.reciprocal(rec, smp)
                t16 = p_work.tile([P, 512], F16, tag="t16")
                if qt == NQT - 1 and ch == 3:
                    # last head before the final output projection: emit the
                    # normalization in 128-col pieces on DVE so the first
                    # final o_proj chunks start before the whole head is done
                    for pc in range(4):
                        c = ds(pc * P, P)
                        nc.vector.tensor_mul(t16[:, c], att[:, c], rec[:, c])
                        nc.vector.tensor_copy(
                            atth[:, ch, ds(qt * 512 + pc * P, P)], t16[:, c])
                        nc.vector.tensor_sub(
                            attl[:, ch, ds(qt * 512 + pc * P, P)], t16[:, c],
                            atth[:, ch, ds(qt * 512 + pc * P, P)])
                else:
                    nc.vector.tensor_mul(t16, att, rec)
                    nc.gpsimd.tensor_copy(atth[:, ch, ts(qt, 512)], t16)
                    nc.gpsimd.tensor_sub(attl[:, ch, ts(qt, 512)], t16,
                                         atth[:, ch, ts(qt, 512)])
                del fin[ch]

            for h in range(4):
                state[h] = (ps3.tile([P, 512], F32, tag="att", name="att"),
                            p_work.tile([P, 512], F16, tag="colsum",
                                        name="colsum"))
                for kt in range(nk):
                    if kt == min(4, nk - 2) and qt > 0:
                        # previous q-tile's output projection emitted mid-head
                        o_proj_chunk(qt - 1, h)
                    j = kt - 4 * qt
                    x0 = j * P if j >= 0 else 0
                    F = 512 - x0
                    sp = ps_s.tile([P, 512], F32, tag="s")
                    nc.tensor.matmul(
                        sp[:, x0:512],
                        lhsT=qkvb[:, 4, ds(kt * P, P)],
                        rhs=qkvb[:, h, ds(qt * 512 + x0, F)],
                        start=True, stop=True,
                    )
                    if kt % 4 == 0:
                        pr_quad = p_probs.tile([P, 4, 512], BF16, tag="probs")
                    pr = pr_quad[:, kt % 4, :]
                    nc.scalar.activation(
                        pr[:, x0:512], sp[:, x0:512],
                        mybir.ActivationFunctionType.Exp,
                        scale=EXP_SCALE, bias=ebias)
                    if j >= 0:
                        # zero the k>q half of the diagonal tile in place
                        # (local col c vs partition p: keep iff c >= p)
                        nc.gpsimd.affine_select(
                            out=pr[:, x0:512], in_=pr[:, x0:512],
                            pattern=[[1, F]], base=0,
                            channel_multiplier=-1,
                            compare_op=mybir.AluOpType.is_ge, fill=0.0)
                    pend.append((h, pr, x0, kt))
                    if len(pend) > LAG:
                        consume()
                        for e in fin_q:
                            e[1] += 1
                        if fin_q and fin_q[0][1] >= 2:
                            finalize(fin_q.popleft()[0])
            while pend:
                consume()
            while fin_q:
                finalize(fin_q.popleft()[0])

        # ------- Fused phases: per token tile: projection+conv, then the
        # attention q-tile that just became computable, then the (pipelined)
        # output projection of the previous q-tile.
        for tt in range(NTT):
            ht_hi = p_ht.tile([P, NPR, 2, 512], FP8, tag="hth")
            ht_lo = p_ht.tile([P, NPR, 2, 512], FP8, tag="htl")
            if tt == 0:
                # weights stream on the ACT hwdge queue (idle at startup), in
                # the same per-fc order the projection chains consume them;
                # hidden-state chunks stream in parallel on the SP queue
                for fc in (4, 5, 0, 1, 2, 3):
                    nc.scalar.dma_start(wq_hi[:, fc], wh_d[:, fc])
                    nc.scalar.dma_start(wq_lo[:, fc], wl_d[:, fc])
            # tile-major host layout -> fully contiguous lines; two large
            # DMAs per tensor (dispatch cost is per-descriptor, so fewer,
            # bigger transfers beat fine-grained chunking)
            for half in range(2):
                nc.sync.dma_start(ht_hi[:, ds(half * 4, 4)],
                                  hh_d[:, tt, ds(half * 4, 4)])
                nc.sync.dma_start(ht_lo[:, ds(half * 4, 4)],
                                  hl_d[:, tt, ds(half * 4, 4)])
            t0 = tt * 512

            def proj_group(pp, fc, g):
                lhs, rhs_h = ((wq_hi, ht_hi), (wq_hi, ht_lo),
                              (wq_lo, ht_hi))[g]
                for pr_ in range(NPR):
                    nc.tensor.matmul(
                        pp, lhsT=lhs[:, fc, pr_], rhs=rhs_h[:, pr_],
                        start=(g == 0 and pr_ == 0),
                        stop=(g == 2 and pr_ == NPR - 1), perf_mode=DR)

            def conv_fc(fc, pp):
                # pre-conv x (64x) -> bf16 for the DVE conv taps
                nc.scalar.copy(qkvf[:, fc, ds(3 + t0, 512)], pp)
                # conv taps: out[t] = x[t] + sum_k x[t+k-3]*w[k].
                # Products via tensor_scalar (4x DVE mode — the tensor-tensor
                # variant gets no fast mode), sums via bf16 tensor_tensor (2x)
                ca = p_work.tile([P, 512], BF16, tag="ctmpa", name="ca")
                cb = p_work.tile([P, 512], BF16, tag="ctmpb", name="cb")
                nc.vector.tensor_scalar(
                    ca, qkvf[:, fc, ds(t0 + 0, 512)],
                    cw[:, fc * 4 + 0: fc * 4 + 1], None, op0=MULT)
                nc.vector.tensor_scalar(
                    cb, qkvf[:, fc, ds(t0 + 1, 512)],
                    cw[:, fc * 4 + 1: fc * 4 + 2], None, op0=MULT)
                nc.vector.tensor_add(ca, ca, cb)
                nc.vector.tensor_scalar(
                    cb, qkvf[:, fc, ds(t0 + 2, 512)],
                    cw[:, fc * 4 + 2: fc * 4 + 3], None, op0=MULT)
                nc.vector.tensor_add(ca, ca, cb)
                nc.vector.tensor_scalar(
                    cb, qkvf[:, fc, ds(t0 + 3, 512)],
                    cw[:, fc * 4 + 3: fc * 4 + 4], None, op0=MULT)
                nc.vector.tensor_add(cb, cb, qkvf[:, fc, ds(t0 + 3, 512)])
                nc.vector.tensor_add(qkvb[:, fc, ts(tt, 512)], ca, cb)

            for fc in (4, 5, 0, 1, 2, 3):
                pp = ps2.tile([P, 512], F32, tag="proj", name="pp")
                for g in range(3):
                    proj_group(pp, fc, g)
                conv_fc(fc, pp)
            # v (fc=5) of this token tile -> natural [token, dh] layout
            trp = ps1.tile([P, 512], BF16, tag="small")
            for j in range(4):
                nc.tensor.transpose(trp[:, ds(j * P, P)],
                                    qkvb[:, 5, ds((tt * 4 + j) * P, P)],
                                    ident)
            nc.vector.tensor_copy(vnat[:, ds(tt * 4, 4), :], trp)
            if tt == 0:
                # w_o load deferred past the critical head DMAs
                nc.sync.dma_start(wo_hi, oh_d)
                nc.sync.dma_start(wo_lo, ol_d)
            attn_B(tt)
        for t4 in range(4):
            o_proj_chunk(NQT - 1, t4, final=True)
        if debug:
            nc.sync.dma_start(dbg_qkvb, qkvb)
            nc.sync.dma_start(dbg_atth, atth)
            nc.sync.dma_start(dbg_qkvf8, qkvf)

    if legalize:
        _legalize_waits(nc)
    _CACHE[key] = nc
    return nc


def _prep_inputs(hidden_states, w_q, w_k, w_v, w_o, conv_w):
    """Build the 8 per-core input maps (host-side shard + fp8 hi/lo split)."""
    f8 = ml_dtypes.float8_e4m3

    def hpairs(x):  # [2048 d, 2048 t] -> [128, 4, 8, 2, 512]
        return np.ascontiguousarray(
            x.reshape(NPR, 2, P, NTT, 512).transpose(2, 3, 0, 1, 4))

    def wpairs(x):  # [2048, 768] -> [128, 6, 8, 2, 128]
        return np.ascontiguousarray(
            x.reshape(NPR, 2, P, NF, P).transpose(2, 3, 0, 1, 4))

    def split8(x):
        hi = x.astype(f8)
        lo = (x - hi.astype(np.float32)).astype(f8)
        return hi, lo

    # hidden split is shared by the 4 cores of a batch
    h_pairs = []
    for b in range(2):
        hT = np.ascontiguousarray(hidden_states[b].T)
        hi, lo = split8(hT)
        h_pairs.append((hpairs(hi), hpairs(lo)))

    in_maps = []
    for c in range(8):
        b, g = c // 4, c % 4
        wqkv = np.concatenate(
            [w_q[:, g * 512:(g + 1) * 512],
             w_k[:, g * 128:(g + 1) * 128],
             w_v[:, g * 128:(g + 1) * 128]], axis=1) * WSCALE
        w_hi, w_lo = split8(wqkv)
        wo = np.ascontiguousarray(w_o[g * 512:(g + 1) * 512, :]) * WSCALE
        wo_hi, wo_lo = split8(wo)
        wo_hi = np.ascontiguousarray(
            wo_hi.reshape(2, 2, P, D).transpose(2, 0, 1, 3))
        wo_lo = np.ascontiguousarray(
            wo_lo.reshape(2, 2, P, D).transpose(2, 0, 1, 3))
        cw = np.concatenate(
            [conv_w[g * 512:(g + 1) * 512],
             conv_w[2048 + g * 128: 2048 + (g + 1) * 128],
             conv_w[2560 + g * 128: 2560 + (g + 1) * 128]], axis=0)  # [768,4]
        cwp = np.ascontiguousarray(
            cw.reshape(NF, P, 4).transpose(1, 0, 2).reshape(P, NF * 4)
        ).astype(np.float32)
        in_maps.append({
            "h_hi": h_pairs[b][0], "h_lo": h_pairs[b][1],
            "w_hi": wpairs(w_hi), "w_lo": wpairs(w_lo),
            "wo_hi": wo_hi, "wo_lo": wo_lo,
            "conv_w": cwp,
        })
    return in_maps


def kernel(hidden_states, w_q, w_k, w_v, w_o, conv_w, _trace=False):
    nc = _build()
    in_maps = _prep_inputs(
        np.asarray(hidden_states, dtype=np.float32),
        np.asarray(w_q, dtype=np.float32),
        np.asarray(w_k, dtype=np.float32),
        np.asarray(w_v, dtype=np.float32),
        np.asarray(w_o, dtype=np.float32),
        np.asarray(conv_w, dtype=np.float32),
    )
    res = run_bass_kernel_spmd(nc, in_maps, core_ids=list(range(8)),
                               trace=_trace)
    outs = [r["out"] for r in res.results]
    full = np.empty((2, S, D), dtype=np.float32)
    for b in range(2):
        acc = outs[4 * b] + outs[4 * b + 1] + outs[4 * b + 2] + outs[4 * b + 3]
        full[b] = acc * (1.0 / OUT_DIV)
    if _trace:
        kernel.last_results = res
    return full


# revision 55
# speedup vs baseline: 1.1742x; 1.0031x over previous
"""CanonCausalMultiheadAttn Trainium2 kernel (fp8 DoubleRow version).

Sharding: 8 cores = 2 (batch) x 4 (kv-head groups). Core c handles batch
c//4 and kv-group g=c%4 (q heads 4g..4g+3, kv head g). w_q/w_k/w_v are
column-sharded by head group, w_o row-sharded; each core emits a partial
[S, D] output which the host sums over the 4 groups of its batch.

The four heads of each q-tile are software-pipelined into one flat
(head, k-tile) sequence with a deep (LAG=10) probs queue, so the in-order
PE queue never stalls on the exp (ACT) -> causal-mask (Pool) producer
chain; the previous q-tile's output projection is interleaved as PE
filler. DMA dispatch cost is per-descriptor, so hidden states use a
tile-major host layout (fully contiguous lines, 2 large DMAs per tile)
and weights stream per-fc on the otherwise-idle ACT hwdge queue.

Per-core dataflow (transposed [feature, token] layout; v transposed on PE):
  qkvT[f, t] = w_qkv[:, f].T @ hT[:, t]   -- fp8e4m3 DoubleRow matmuls with
      3-term hi/lo compensation (w_hi.h_hi + w_lo.h_hi + w_hi.h_lo), which
      matches bf16 accuracy at 0.75x the PE time (DR = 0.5 cy/row, 256-deep
      contraction per instruction).
  conv: depthwise causal taps in bf16 on DVE: per-tap products via
      tensor_scalar (4x DVE mode), summed with bf16 tensor_tensor adds
      (2x mode); weights stay near-exact (fp8 conv weights measured 5e-3
      end-to-end error, so the taps are NOT quantized).
  scores.T[k, q] = kT.T @ qT (bf16) -> exp on ACT (scale folds the fp8
      pre-scales; bias -2ln2 keeps fp16 column sums in range)
  causal: k-tiles with k0 <= q_end only; diagonal tiles masked in-place
      by an affine_select on the (otherwise idle) Pool engine.
  attT[dh, q] += v_nat[k,:].T @ probsT  (bf16)
  colsum via DVE adds in fp16 (2x DVE mode), partition-reduced by one
      fp16 ones-matmul (ones=4.0 folds the attT scale correction).
  attT stored as fp8 hi+lo; out[t, d] = attT.T @ w_o_rows via 3-term DR.
  Host divides the gathered partial sums by the net 2048x scale.

Scales: w_qkv and w_o are pre-scaled x64 into fp8 (e4m3 = IEEE variant,
  max finite 240); hidden stays x1. qkv = 64x, scores = 4096 s (folded
  into exp scale), probs = p/4 (exp bias), att = 16*Sum p v, colsum = p/4
  summed, ones=4.0 => attT = 16*attended, out = 1024*true. Host divides.
"""

import numpy as np
import ml_dtypes
from collections import deque
from contextlib import ExitStack

import concourse.bass as bass
import concourse.tile as tile
import concourse.mybir as mybir
from concourse.bass import ds, ts
from concourse.bass_utils import run_bass_kernel_spmd
from concourse.masks import make_identity

BF16 = mybir.dt.bfloat16
F16 = mybir.dt.float16
F32 = mybir.dt.float32
FP8 = mybir.dt.float8e4
DR = mybir.MatmulPerfMode.DoubleRow
P = 128
S = 2048          # sequence length
D = 2048          # d_model
NF = 6            # feature chunks of 128: 4 q heads, 1 k, 1 v
NPR = 8           # DR contraction pairs over d_model (2048 = 8*256)
NQT = S // 512    # 4 query tiles of 512
NTT = S // 512    # 4 token tiles of 512
WSCALE = 64.0     # fp8 pre-scale on w_qkv and w_o (e4m3 max is 240)
ISQ = 1.0 / np.sqrt(128.0)
EXP_SCALE = ISQ / (WSCALE * WSCALE)   # scores PSUM holds 4096*s
EXP_BIAS = float(-2.0 * np.log(2.0))  # probs = p/4 (fp16 colsum headroom)
OUT_DIV = 1024.0  # 16 (attT) * 64 (w_o)
MULT = mybir.AluOpType.mult
ADD = mybir.AluOpType.add

_CACHE = {}


def _legalize_waits(nc):
    """Split multi-wait sync_info into preceding single-wait engine NOPs.

    The walrus codegen in this container accepts at most ONE sync wait per
    TPB instruction ("Too many sync wait commands"), but the Tile scheduler
    freely emits several. An engine executes its queue in order, so hoisting
    the extra waits onto NoOps right before the instruction is equivalent.
    """
    n = 0
    for f in nc.m.functions:
        for blk in f.blocks:
            out = []
            changed = False
            for inst in blk.instructions:
                si = inst.sync_info
                if (si is not None and si.on_wait and len(si.on_wait) > 1
                        and str(inst.engine) != "EngineType.Unassigned"):
                    waits = list(si.on_wait)
                    for w in waits[:-1]:
                        out.append(mybir.InstNoOp(
                            name=f"I-wf{n}", engine=inst.engine, ins=[],
                            outs=[],
                            sync_info=mybir.SyncInfo(on_wait=[w],
                                                     on_update=[])))
                        n += 1
                    si.on_wait = [waits[-1]]
                    changed = True
                out.append(inst)
            if changed:
                blk.instructions = out
    return n


def _build(legalize=True, debug=False):
    key = ("nc" if legalize else "nc_raw") + ("_dbg" if debug else "")
    if key in _CACHE:
        return _CACHE[key]
    nc = bass.Bass("TRN2", target_bir_lowering=False, debug=False)

    hh_d = nc.dram_tensor("h_hi", [P, NTT, NPR, 2, 512], FP8, kind="ExternalInput").ap()
    hl_d = nc.dram_tensor("h_lo", [P, NTT, NPR, 2, 512], FP8, kind="ExternalInput").ap()
    wh_d = nc.dram_tensor("w_hi", [P, NF, NPR, 2, P], FP8, kind="ExternalInput").ap()
    wl_d = nc.dram_tensor("w_lo", [P, NF, NPR, 2, P], FP8, kind="ExternalInput").ap()
    oh_d = nc.dram_tensor("wo_hi", [P, 2, 2, D], FP8, kind="ExternalInput").ap()
    ol_d = nc.dram_tensor("wo_lo", [P, 2, 2, D], FP8, kind="ExternalInput").ap()
    cw_d = nc.dram_tensor("conv_w", [P, NF * 4], F32, kind="ExternalInput").ap()
    out_d = nc.dram_tensor("out", [S, D], F32, kind="ExternalOutput").ap()
    if debug:
        dbg_qkvb = nc.dram_tensor("dbg_qkvb", [P, NF, S], BF16,
                                  kind="ExternalOutput").ap()
        dbg_atth = nc.dram_tensor("dbg_atth", [P, 4, S], FP8,
                                  kind="ExternalOutput").ap()
        dbg_qkvf8 = nc.dram_tensor("dbg_qkvf8", [P, NF, S + 3], BF16,
                                   kind="ExternalOutput").ap()

    out_v = out_d.rearrange("(po pi) d -> pi po d", pi=P)      # [128,16,2048]

    with tile.TileContext(nc) as tc, ExitStack() as ctx:
        const = ctx.enter_context(tc.tile_pool(name="const", bufs=1))
        p_ht = ctx.enter_context(tc.tile_pool(name="ht", bufs=2))
        p_work = ctx.enter_context(tc.tile_pool(name="work", bufs=2))
        p_probs = ctx.enter_context(tc.tile_pool(name="probs", bufs=4))
        p_out = ctx.enter_context(tc.tile_pool(name="outp", bufs=8))
        ps2 = ctx.enter_context(tc.tile_pool(name="ps2", bufs=2, space="PSUM"))
        ps_s = ctx.enter_context(tc.tile_pool(name="ps_s", bufs=3, space="PSUM"))
        ps3 = ctx.enter_context(tc.tile_pool(name="ps3", bufs=2, space="PSUM"))
        ps1 = ctx.enter_context(tc.tile_pool(name="ps1", bufs=1, space="PSUM"))

        # --- constants / persistent tensors ---
        ident = const.tile([P, P], BF16, tag="ident")
        make_identity(nc, ident)
        cw0 = const.tile([P, NF * 4], F32, tag="cw0")
        nc.scalar.dma_start(cw0, cw_d)
        # conv ops read cw via a DVE copy so their DMA wait lands here, not
        # on the (wait-slot-limited) Pool scalar_tensor_tensor instructions
        cw = const.tile([P, NF * 4], F32, tag="cw")
        nc.vector.tensor_copy(cw, cw0)
        wq_hi = const.tile([P, NF, NPR, 2, P], FP8, tag="wqh")
        wq_lo = const.tile([P, NF, NPR, 2, P], FP8, tag="wql")
        wo_hi = const.tile([P, 2, 2, D], FP8, tag="woh")
        wo_lo = const.tile([P, 2, 2, D], FP8, tag="wol")
        # raw (pre-conv) qkv.T in bf16 (64x scale), 3 leading zero columns so
        # the causal conv taps can read t-3..t-1 without edge cases
        qkvf = const.tile([P, NF, S + 3], BF16, tag="qkvf")
        nc.gpsimd.memset(qkvf[:, :, 0:3], 0.0)
        qkvb = const.tile([P, NF, S], BF16, tag="qkvb")    # conv'd qkv.T
        vnat = const.tile([P, 16, P], BF16, tag="vnat")    # v in [token, dh]
        atth = const.tile([P, 4, S], FP8, tag="atth")      # attT hi per head
        attl = const.tile([P, 4, S], FP8, tag="attl")      # attT lo per head
        ones2 = const.tile([P, P], F16, tag="ones2")
        nc.vector.memset(ones2, 4.0)
        ebias = const.tile([P, 1], F32, tag="ebias")
        nc.vector.memset(ebias, EXP_BIAS)

        def o_proj_chunk(qt, t4, final=False):
            # output projection for one token-128-tile of q-tile qt
            tt16 = qt * 4 + t4
            for dt in range(4):
                op = ps2.tile([P, 512], F32, tag="proj")
                k = 0
                for lhs, rhs_w in ((atth, wo_hi), (attl, wo_hi), (atth, wo_lo)):
                    for pr_ in range(2):
                        nc.tensor.matmul(
                            op,
                            lhsT=lhs[:, ds(2 * pr_, 2), ds(tt16 * P, P)],
                            rhs=rhs_w[:, pr_, :, ds(dt * 512, 512)],
                            start=(k == 0), stop=(k == 5), perf_mode=DR)
                        k += 1
                ob = p_out.tile([P, 512], F32, tag="ob")
                if dt == 3 or final:
                    nc.scalar.copy(ob, op)
                else:
                    nc.vector.tensor_copy(ob, op)
                nc.sync.dma_start(out_v[:, tt16, ds(dt * 512, 512)], ob)

        def attn_B(qt):
            # attention for q-tile qt (needs token tiles <= qt). The four
            # heads are software-pipelined into ONE flat (h, kt) sequence:
            # scores of head h+1 are emitted while head h's attended matmuls
            # drain, so the in-order PE queue never stalls on the
            # exp (ACT) -> mask (Pool) producer chain. The previous q-tile's
            # output projection is interleaved as additional PE filler.
            nk = 4 * (qt + 1)
            LAG = 10
            state = {}  # h -> (att, colsum)
            fin = {}    # h -> (att, colsum) awaiting denominator finalize
            fin_q = deque()  # [h, consumes-since-ready]
            pend = deque()
            pr_quad = None

            def consume():
                ch, ppr, px0, pkt = pend.popleft()
                att, colsum = state[ch] if ch in state else fin[ch]
                nc.tensor.matmul(
                    att[:, px0:512], lhsT=vnat[:, pkt, :],
                    rhs=ppr[:, px0:512],
                    start=(pkt == 0), stop=(pkt == nk - 1))
                # softmax denominator: accumulate exp'd probs on DVE
                # (partition dim reduced by ONE ones-matmul at the end)
                if pkt == 0:
                    nc.vector.tensor_copy(colsum, ppr)
                else:
                    nc.vector.tensor_add(
                        colsum[:, px0:512], colsum[:, px0:512],
                        ppr[:, px0:512])
                if pkt == nk - 1:
                    fin[ch] = state.pop(ch)
                    fin_q.append([ch, 0])

            def finalize(ch):
                att, colsum = fin[ch]
                smp = ps1.tile([P, 512], F32, tag="small")
                nc.tensor.matmul(smp, lhsT=ones2, rhs=colsum,
                                 start=True, stop=True)
                rec = p_work.tile([P, 512], F32, tag="rec")
                nc.vector.reciprocal(rec, smp)
                t16 = p_work.tile([P, 512], F16, tag="t16")
                if qt == NQT - 1 and ch == 3:
                    # last head before the final output projection: emit the
                    # normalization in 128-col pieces on DVE so the first
                    # final o_proj chunks start before the whole head is done
                    for pc in range(4):
                        c = ds(pc * P, P)
                        nc.vector.tensor_mul(t16[:, c], att[:, c], rec[:, c])
                        nc.vector.tensor_copy(
                            atth[:, ch, ds(qt * 512 + pc * P, P)], t16[:, c])
                        nc.vector.tensor_sub(
                            attl[:, ch, ds(qt * 512 + pc * P, P)], t16[:, c],
                            atth[:, ch, ds(qt * 512 + pc * P, P)])
                else:
                    nc.vector.tensor_mul(t16, att, rec)
                    nc.gpsimd.tensor_copy(atth[:, ch, ts(qt, 512)], t16)
                    nc.gpsimd.tensor_sub(attl[:, ch, ts(qt, 512)], t16,
                                         atth[:, ch, ts(qt, 512)])
                del fin[ch]

            for h in range(4):
                state[h] = (ps3.tile([P, 512], F32, tag="att", name="att"),
                            p_work.tile([P, 512], F16, tag="colsum",
                                        name="colsum"))
                for kt in range(nk):
                    if kt == min(4, nk - 2) and qt > 0:
                        # previous q-tile's output projection emitted mid-head
                        o_proj_chunk(qt - 1, h)
                    j = kt - 4 * qt
                    x0 = j * P if j >= 0 else 0
                    F = 512 - x0
                    sp = ps_s.tile([P, 512], F32, tag="s")
                    nc.tensor.matmul(
                        sp[:, x0:512],
                        lhsT=qkvb[:, 4, ds(kt * P, P)],
                        rhs=qkvb[:, h, ds(qt * 512 + x0, F)],
                        start=True, stop=True,
                    )
                    if kt % 4 == 0:
                        pr_quad = p_probs.tile([P, 4, 512], BF16, tag="probs")
                    pr = pr_quad[:, kt % 4, :]
                    nc.scalar.activation(
                        pr[:, x0:512], sp[:, x0:512],
                        mybir.ActivationFunctionType.Exp,
                        scale=EXP_SCALE, bias=ebias)
                    if j >= 0:
                        # zero the k>q half of the diagonal tile in place
                        # (local col c vs partition p: keep iff c >= p)
                        nc.gpsimd.affine_select(
                            out=pr[:, x0:512], in_=pr[:, x0:512],
                            pattern=[[1, F]], base=0,
                            channel_multiplier=-1,
                            compare_op=mybir.AluOpType.is_ge, fill=0.0)
                    pend.append((h, pr, x0, kt))
                    if len(pend) > LAG:
                        consume()
                        for e in fin_q:
                            e[1] += 1
                        if fin_q and fin_q[0][1] >= 2:
                            finalize(fin_q.popleft()[0])
            while pend:
                consume()
            while fin_q:
                finalize(fin_q.popleft()[0])

        # ------- Fused phases: per token tile: projection+conv, then the
        # attention q-tile that just became computable, then the (pipelined)
        # output projection of the previous q-tile.
        for tt in range(NTT):
            ht_hi = p_ht.tile([P, NPR, 2, 512], FP8, tag="hth")
            ht_lo = p_ht.tile([P, NPR, 2, 512], FP8, tag="htl")
            if tt == 0:
                # weights stream on the ACT hwdge queue (idle at startup), in
                # the same per-fc order the projection chains consume them;
                # hidden-state chunks stream in parallel on the SP queue
                for fc in (4, 5, 0, 1, 2, 3):
                    nc.scalar.dma_start(wq_hi[:, fc], wh_d[:, fc])
                    nc.scalar.dma_start(wq_lo[:, fc], wl_d[:, fc])
            # tile-major host layout -> fully contiguous lines; two large
            # DMAs per tensor (dispatch cost is per-descriptor, so fewer,
            # bigger transfers beat fine-grained chunking)
            for half in range(2):
                nc.sync.dma_start(ht_hi[:, ds(half * 4, 4)],
                                  hh_d[:, tt, ds(half * 4, 4)])
                nc.sync.dma_start(ht_lo[:, ds(half * 4, 4)],
                                  hl_d[:, tt, ds(half * 4, 4)])
            t0 = tt * 512

            def proj_group(pp, fc, g):
                lhs, rhs_h = ((wq_hi, ht_hi), (wq_hi, ht_lo),
                              (wq_lo, ht_hi))[g]
                for pr_ in range(NPR):
                    nc.tensor.matmul(
                        pp, lhsT=lhs[:, fc, pr_], rhs=rhs_h[:, pr_],
                        start=(g == 0 and pr_ == 0),
                        stop=(g == 2 and pr_ == NPR - 1), perf_mode=DR)

            def conv_fc(fc, pp):
                # pre-conv x (64x) -> bf16 for the DVE conv taps
                nc.scalar.copy(qkvf[:, fc, ds(3 + t0, 512)], pp)
                # conv taps: out[t] = x[t] + sum_k x[t+k-3]*w[k].
                # Products via tensor_scalar (4x DVE mode — the tensor-tensor
                # variant gets no fast mode), sums via bf16 tensor_tensor (2x)
                ca = p_work.tile([P, 512], BF16, tag="ctmpa", name="ca")
                cb = p_work.tile([P, 512], BF16, tag="ctmpb", name="cb")
                nc.vector.tensor_scalar(
                    ca, qkvf[:, fc, ds(t0 + 0, 512)],
                    cw[:, fc * 4 + 0: fc * 4 + 1], None, op0=MULT)
                nc.vector.tensor_scalar(
                    cb, qkvf[:, fc, ds(t0 + 1, 512)],
                    cw[:, fc * 4 + 1: fc * 4 + 2], None, op0=MULT)
                nc.vector.tensor_add(ca, ca, cb)
                nc.vector.tensor_scalar(
                    cb, qkvf[:, fc, ds(t0 + 2, 512)],
                    cw[:, fc * 4 + 2: fc * 4 + 3], None, op0=MULT)
                nc.vector.tensor_add(ca, ca, cb)
                nc.vector.tensor_scalar(
                    cb, qkvf[:, fc, ds(t0 + 3, 512)],
                    cw[:, fc * 4 + 3: fc * 4 + 4], None, op0=MULT)
                nc.vector.tensor_add(cb, cb, qkvf[:, fc, ds(t0 + 3, 512)])
                nc.vector.tensor_add(qkvb[:, fc, ts(tt, 512)], ca, cb)

            for fc in (4, 5, 0, 1, 2, 3):
                pp = ps2.tile([P, 512], F32, tag="proj", name="pp")
                for g in range(3):
                    proj_group(pp, fc, g)
                conv_fc(fc, pp)
            # v (fc=5) of this token tile -> natural [token, dh] layout
            trp = ps1.tile([P, 512], BF16, tag="small")
            for j in range(4):
                nc.tensor.transpose(trp[:, ds(j * P, P)],
                                    qkvb[:, 5, ds((tt * 4 + j) * P, P)],
                                    ident)
            nc.vector.tensor_copy(vnat[:, ds(tt * 4, 4), :], trp)
            if tt == 0:
                # w_o load deferred past the critical head DMAs
                nc.sync.dma_start(wo_hi, oh_d)
                nc.sync.dma_start(wo_lo, ol_d)
            attn_B(tt)
        for t4 in range(4):
            o_proj_chunk(NQT - 1, t4, final=True)
        if debug:
            nc.sync.dma_start(dbg_qkvb, qkvb)
            nc.sync.dma_start(dbg_atth, atth)
            nc.sync.dma_start(dbg_qkvf8, qkvf)

    if legalize:
        _legalize_waits(nc)
    _CACHE[key] = nc
    return nc


def _prep_inputs(hidden_states, w_q, w_k, w_v, w_o, conv_w):
    """Build the 8 per-core input maps (host-side shard + fp8 hi/lo split)."""
    f8 = ml_dtypes.float8_e4m3

    def hpairs(x):  # [2048 d, 2048 t] -> [128, 4, 8, 2, 512]
        return np.ascontiguousarray(
            x.reshape(NPR, 2, P, NTT, 512).transpose(2, 3, 0, 1, 4))

    def wpairs(x):  # [2048, 768] -> [128, 6, 8, 2, 128]
        return np.ascontiguousarray(
            x.reshape(NPR, 2, P, NF, P).transpose(2, 3, 0, 1, 4))

    def split8(x):
        hi = x.astype(f8)
        lo = (x - hi.astype(np.float32)).astype(f8)
        return hi, lo

    # hidden split is shared by the 4 cores of a batch
    h_pairs = []
    for b in range(2):
        hT = np.ascontiguousarray(hidden_states[b].T)
        hi, lo = split8(hT)
        h_pairs.append((hpairs(hi), hpairs(lo)))

    in_maps = []
    for c in range(8):
        b, g = c // 4, c % 4
        wqkv = np.concatenate(
            [w_q[:, g * 512:(g + 1) * 512],
             w_k[:, g * 128:(g + 1) * 128],
             w_v[:, g * 128:(g + 1) * 128]], axis=1) * WSCALE
        w_hi, w_lo = split8(wqkv)
        wo = np.ascontiguousarray(w_o[g * 512:(g + 1) * 512, :]) * WSCALE
        wo_hi, wo_lo = split8(wo)
        wo_hi = np.ascontiguousarray(
            wo_hi.reshape(2, 2, P, D).transpose(2, 0, 1, 3))
        wo_lo = np.ascontiguousarray(
            wo_lo.reshape(2, 2, P, D).transpose(2, 0, 1, 3))
        cw = np.concatenate(
            [conv_w[g * 512:(g + 1) * 512],
             conv_w[2048 + g * 128: 2048 + (g + 1) * 128],
             conv_w[2560 + g * 128: 2560 + (g + 1) * 128]], axis=0)  # [768,4]
        cwp = np.ascontiguousarray(
            cw.reshape(NF, P, 4).transpose(1, 0, 2).reshape(P, NF * 4)
        ).astype(np.float32)
        in_maps.append({
            "h_hi": h_pairs[b][0], "h_lo": h_pairs[b][1],
            "w_hi": wpairs(w_hi), "w_lo": wpairs(w_lo),
            "wo_hi": wo_hi, "wo_lo": wo_lo,
            "conv_w": cwp,
        })
    return in_maps


def kernel(hidden_states, w_q, w_k, w_v, w_o, conv_w, _trace=False):
    nc = _build()
    in_maps = _prep_inputs(
        np.asarray(hidden_states, dtype=np.float32),
        np.asarray(w_q, dtype=np.float32),
        np.asarray(w_k, dtype=np.float32),
        np.asarray(w_v, dtype=np.float32),
        np.asarray(w_o, dtype=np.float32),
        np.asarray(conv_w, dtype=np.float32),
    )
    res = run_bass_kernel_spmd(nc, in_maps, core_ids=list(range(8)),
                               trace=_trace)
    outs = [r["out"] for r in res.results]
    full = np.empty((2, S, D), dtype=np.float32)
    for b in range(2):
        acc = outs[4 * b] + outs[4 * b + 1] + outs[4 * b + 2] + outs[4 * b + 3]
        full[b] = acc * (1.0 / OUT_DIV)
    if _trace:
        kernel.last_results = res
    return full


# revision 62
# speedup vs baseline: 1.1819x; 1.0066x over previous
"""CanonCausalMultiheadAttn Trainium2 kernel (fp8 DoubleRow version).

Sharding: 8 cores = 2 (batch) x 4 (kv-head groups). Core c handles batch
c//4 and kv-group g=c%4 (q heads 4g..4g+3, kv head g). w_q/w_k/w_v are
column-sharded by head group, w_o row-sharded; each core emits a partial
[S, D] output which the host sums over the 4 groups of its batch.

The four heads of each q-tile are software-pipelined into one flat
(head, k-tile) sequence with a deep (LAG=10) pending-probs queue, so the in-order
PE queue never stalls on the exp (ACT) -> causal-mask (Pool) producer
chain; the previous q-tile's output projection is interleaved as PE
filler. DMA dispatch cost is per-descriptor, so hidden states use a
tile-major host layout (fully contiguous lines, 2 large DMAs per tile)
and weights stream per-fc on the otherwise-idle ACT hwdge queue.

Per-core dataflow (transposed [feature, token] layout; v transposed on PE):
  qkvT[f, t] = w_qkv[:, f].T @ hT[:, t]   -- fp8e4m3 DoubleRow matmuls with
      3-term hi/lo compensation (w_hi.h_hi + w_lo.h_hi + w_hi.h_lo), which
      matches bf16 accuracy at 0.75x the PE time (DR = 0.5 cy/row, 256-deep
      contraction per instruction).
  conv: depthwise causal taps in bf16 on DVE: per-tap products via
      tensor_scalar (4x DVE mode), summed with bf16 tensor_tensor adds
      (2x mode); weights stay near-exact (fp8 conv weights measured 5e-3
      end-to-end error, so the taps are NOT quantized).
  scores.T[k, q] = kT.T @ qT (bf16) -> exp on ACT (scale folds the fp8
      pre-scales; bias -2ln2 keeps fp16 column sums in range)
  causal: k-tiles with k0 <= q_end only; diagonal tiles masked in-place
      by an affine_select on the (otherwise idle) Pool engine.
  attT[dh, q] += v_nat[k,:].T @ probsT  (bf16)
  colsum via DVE adds in fp16 (2x DVE mode), partition-reduced by one
      fp16 ones-matmul (ones=4.0 folds the attT scale correction).
  attT stored as fp8 hi+lo; out[t, d] = attT.T @ w_o_rows via 3-term DR.
  Host divides the gathered partial sums by the net 2048x scale.

Scales: w_qkv and w_o are pre-scaled x64 into fp8 (e4m3 = IEEE variant,
  max finite 240); hidden stays x1. qkv = 64x, scores = 4096 s (folded
  into exp scale), probs = p/4 (exp bias), att = 16*Sum p v, colsum = p/4
  summed, ones=4.0 => attT = 16*attended, out = 1024*true. Host divides.
"""

import numpy as np
import ml_dtypes
from collections import deque
from contextlib import ExitStack

import concourse.bass as bass
import concourse.tile as tile
import concourse.mybir as mybir
from concourse.bass import ds, ts
from concourse.bass_utils import run_bass_kernel_spmd
from concourse.masks import make_identity

BF16 = mybir.dt.bfloat16
F16 = mybir.dt.float16
F32 = mybir.dt.float32
FP8 = mybir.dt.float8e4
DR = mybir.MatmulPerfMode.DoubleRow
P = 128
S = 2048          # sequence length
D = 2048          # d_model
NF = 6            # feature chunks of 128: 4 q heads, 1 k, 1 v
NPR = 8           # DR contraction pairs over d_model (2048 = 8*256)
NQT = S // 512    # 4 query tiles of 512
NTT = S // 512    # 4 token tiles of 512
WSCALE = 64.0     # fp8 pre-scale on w_qkv and w_o (e4m3 max is 240)
ISQ = 1.0 / np.sqrt(128.0)
EXP_SCALE = ISQ / (WSCALE * WSCALE)   # scores PSUM holds 4096*s
EXP_BIAS = float(-2.0 * np.log(2.0))  # probs = p/4 (fp16 colsum headroom)
OUT_DIV = 1024.0  # 16 (attT) * 64 (w_o)
MULT = mybir.AluOpType.mult
ADD = mybir.AluOpType.add

_CACHE = {}


def _legalize_waits(nc):
    """Split multi-wait sync_info into preceding single-wait engine NOPs.

    The walrus codegen in this container accepts at most ONE sync wait per
    TPB instruction ("Too many sync wait commands"), but the Tile scheduler
    freely emits several. An engine executes its queue in order, so hoisting
    the extra waits onto NoOps right before the instruction is equivalent.
    """
    n = 0
    for f in nc.m.functions:
        for blk in f.blocks:
            out = []
            changed = False
            for inst in blk.instructions:
                si = inst.sync_info
                if (si is not None and si.on_wait and len(si.on_wait) > 1
                        and str(inst.engine) != "EngineType.Unassigned"):
                    waits = list(si.on_wait)
                    for w in waits[:-1]:
                        out.append(mybir.InstNoOp(
                            name=f"I-wf{n}", engine=inst.engine, ins=[],
                            outs=[],
                            sync_info=mybir.SyncInfo(on_wait=[w],
                                                     on_update=[])))
                        n += 1
                    si.on_wait = [waits[-1]]
                    changed = True
                out.append(inst)
            if changed:
                blk.instructions = out
    return n


def _build(legalize=True, debug=False):
    key = ("nc" if legalize else "nc_raw") + ("_dbg" if debug else "")
    if key in _CACHE:
        return _CACHE[key]
    nc = bass.Bass("TRN2", target_bir_lowering=False, debug=False)

    hh_d = nc.dram_tensor("h_hi", [P, NTT, NPR, 2, 512], FP8, kind="ExternalInput").ap()
    hl_d = nc.dram_tensor("h_lo", [P, NTT, NPR, 2, 512], FP8, kind="ExternalInput").ap()
    wh_d = nc.dram_tensor("w_hi", [P, NF, NPR, 2, P], FP8, kind="ExternalInput").ap()
    wl_d = nc.dram_tensor("w_lo", [P, NF, NPR, 2, P], FP8, kind="ExternalInput").ap()
    oh_d = nc.dram_tensor("wo_hi", [P, 2, 2, D], FP8, kind="ExternalInput").ap()
    ol_d = nc.dram_tensor("wo_lo", [P, 2, 2, D], FP8, kind="ExternalInput").ap()
    cw_d = nc.dram_tensor("conv_w", [P, NF * 4], F32, kind="ExternalInput").ap()
    out_d = nc.dram_tensor("out", [S, D], F32, kind="ExternalOutput").ap()
    if debug:
        dbg_qkvb = nc.dram_tensor("dbg_qkvb", [P, NF, S], BF16,
                                  kind="ExternalOutput").ap()
        dbg_atth = nc.dram_tensor("dbg_atth", [P, 4, S], FP8,
                                  kind="ExternalOutput").ap()
        dbg_qkvf8 = nc.dram_tensor("dbg_qkvf8", [P, NF, S + 3], BF16,
                                   kind="ExternalOutput").ap()

    out_v = out_d.rearrange("(po pi) d -> pi po d", pi=P)      # [128,16,2048]

    with tile.TileContext(nc) as tc, ExitStack() as ctx:
        const = ctx.enter_context(tc.tile_pool(name="const", bufs=1))
        p_ht = ctx.enter_context(tc.tile_pool(name="ht", bufs=2))
        p_work = ctx.enter_context(tc.tile_pool(name="work", bufs=2))
        p_probs = ctx.enter_context(tc.tile_pool(name="probs", bufs=4))
        p_out = ctx.enter_context(tc.tile_pool(name="outp", bufs=8))
        ps2 = ctx.enter_context(tc.tile_pool(name="ps2", bufs=2, space="PSUM"))
        ps_s = ctx.enter_context(tc.tile_pool(name="ps_s", bufs=3, space="PSUM"))
        ps3 = ctx.enter_context(tc.tile_pool(name="ps3", bufs=2, space="PSUM"))
        ps1 = ctx.enter_context(tc.tile_pool(name="ps1", bufs=1, space="PSUM"))

        # --- constants / persistent tensors ---
        ident = const.tile([P, P], BF16, tag="ident")
        make_identity(nc, ident)
        cw0 = const.tile([P, NF * 4], F32, tag="cw0")
        nc.scalar.dma_start(cw0, cw_d)
        # conv ops read cw via a DVE copy so their DMA wait lands here, not
        # on the (wait-slot-limited) Pool scalar_tensor_tensor instructions
        cw = const.tile([P, NF * 4], F32, tag="cw")
        nc.vector.tensor_copy(cw, cw0)
        wq_hi = const.tile([P, NF, NPR, 2, P], FP8, tag="wqh")
        wq_lo = const.tile([P, NF, NPR, 2, P], FP8, tag="wql")
        wo_hi = const.tile([P, 2, 2, D], FP8, tag="woh")
        wo_lo = const.tile([P, 2, 2, D], FP8, tag="wol")
        # raw (pre-conv) qkv.T in bf16 (64x scale), 3 leading zero columns so
        # the causal conv taps can read t-3..t-1 without edge cases
        qkvf = const.tile([P, NF, S + 3], BF16, tag="qkvf")
        nc.gpsimd.memset(qkvf[:, :, 0:3], 0.0)
        qkvb = const.tile([P, NF, S], BF16, tag="qkvb")    # conv'd qkv.T
        vnat = const.tile([P, 16, P], BF16, tag="vnat")    # v in [token, dh]
        atth = const.tile([P, 4, S], FP8, tag="atth")      # attT hi per head
        attl = const.tile([P, 4, S], FP8, tag="attl")      # attT lo per head
        ones2 = const.tile([P, P], F16, tag="ones2")
        nc.vector.memset(ones2, 4.0)
        ebias = const.tile([P, 1], F32, tag="ebias")
        nc.vector.memset(ebias, EXP_BIAS)

        def o_proj_chunk(qt, t4, final=False, dts=(0, 1, 2, 3)):
            # output projection for one token-128-tile of q-tile qt
            tt16 = qt * 4 + t4
            for dt in dts:
                if final and dt % 2 == 1:
                    op = ps_s.tile([P, 512], F32, tag="s")
                else:
                    op = ps2.tile([P, 512], F32, tag="proj")
                k = 0
                for lhs, rhs_w in ((atth, wo_hi), (attl, wo_hi), (atth, wo_lo)):
                    for pr_ in range(2):
                        nc.tensor.matmul(
                            op,
                            lhsT=lhs[:, ds(2 * pr_, 2), ds(tt16 * P, P)],
                            rhs=rhs_w[:, pr_, :, ds(dt * 512, 512)],
                            start=(k == 0), stop=(k == 5), perf_mode=DR)
                        k += 1
                ob = p_out.tile([P, 512], F32, tag="ob")
                if dt == 3 or (final and dt % 2 == 0):
                    nc.scalar.copy(ob, op)
                else:
                    nc.vector.tensor_copy(ob, op)
                nc.sync.dma_start(out_v[:, tt16, ds(dt * 512, 512)], ob)

        def attn_B(qt):
            # attention for q-tile qt (needs token tiles <= qt). The four
            # heads are software-pipelined into ONE flat (h, kt) sequence:
            # scores of head h+1 are emitted while head h's attended matmuls
            # drain, so the in-order PE queue never stalls on the
            # exp (ACT) -> mask (Pool) producer chain. The previous q-tile's
            # output projection is interleaved as additional PE filler.
            nk = 4 * (qt + 1)
            LAG = 10
            state = {}  # h -> (att, colsum)
            fin = {}    # h -> (att, colsum) awaiting denominator finalize
            fin_q = deque()  # [h, consumes-since-ready]
            pend = deque()
            pr_quad = None

            def consume():
                ch, ppr, px0, pkt = pend.popleft()
                att, colsum = state[ch] if ch in state else fin[ch]
                nc.tensor.matmul(
                    att[:, px0:512], lhsT=vnat[:, pkt, :],
                    rhs=ppr[:, px0:512],
                    start=(pkt == 0), stop=(pkt == nk - 1))
                # softmax denominator: accumulate exp'd probs on DVE
                # (partition dim reduced by ONE ones-matmul at the end)
                if pkt == 0:
                    nc.vector.tensor_copy(colsum, ppr)
                else:
                    nc.vector.tensor_add(
                        colsum[:, px0:512], colsum[:, px0:512],
                        ppr[:, px0:512])
                if pkt == nk - 1:
                    fin[ch] = state.pop(ch)
                    fin_q.append([ch, 0])

            def finalize(ch):
                att, colsum = fin[ch]
                smp = ps1.tile([P, 512], F32, tag="small")
                nc.tensor.matmul(smp, lhsT=ones2, rhs=colsum,
                                 start=True, stop=True)
                rec = p_work.tile([P, 512], F32, tag="rec")
                nc.vector.reciprocal(rec, smp)
                t16 = p_work.tile([P, 512], F16, tag="t16")
                if qt == NQT - 1 and ch == 3:
                    # last head before the final output projection: emit the
                    # normalization in 128-col pieces on DVE so the first
                    # final o_proj chunks start before the whole head is done
                    for pc in range(4):
                        c = ds(pc * P, P)
                        nc.vector.tensor_mul(t16[:, c], att[:, c], rec[:, c])
                        nc.vector.tensor_copy(
                            atth[:, ch, ds(qt * 512 + pc * P, P)], t16[:, c])
                        nc.vector.tensor_sub(
                            attl[:, ch, ds(qt * 512 + pc * P, P)], t16[:, c],
                            atth[:, ch, ds(qt * 512 + pc * P, P)])
                else:
                    nc.vector.tensor_mul(t16, att, rec)
                    nc.gpsimd.tensor_copy(atth[:, ch, ts(qt, 512)], t16)
                    nc.gpsimd.tensor_sub(attl[:, ch, ts(qt, 512)], t16,
                                         atth[:, ch, ts(qt, 512)])
                del fin[ch]

            for h in range(4):
                state[h] = (ps3.tile([P, 512], F32, tag="att", name="att"),
                            p_work.tile([P, 512], F16, tag="colsum",
                                        name="colsum"))
                for kt in range(nk):
                    if kt == min(4, nk - 2) and qt > 0:
                        # previous q-tile's output projection emitted mid-head
                        o_proj_chunk(qt - 1, h, dts=(0, 1))
                    if kt == min(7, nk - 1) and qt > 0:
                        o_proj_chunk(qt - 1, h, dts=(2, 3))
                    j = kt - 4 * qt
                    x0 = j * P if j >= 0 else 0
                    F = 512 - x0
                    sp = ps_s.tile([P, 512], F32, tag="s")
                    nc.tensor.matmul(
                        sp[:, x0:512],
                        lhsT=qkvb[:, 4, ds(kt * P, P)],
                        rhs=qkvb[:, h, ds(qt * 512 + x0, F)],
                        start=True, stop=True,
                    )
                    if kt % 4 == 0:
                        pr_quad = p_probs.tile([P, 4, 512], BF16, tag="probs")
                    pr = pr_quad[:, kt % 4, :]
                    nc.scalar.activation(
                        pr[:, x0:512], sp[:, x0:512],
                        mybir.ActivationFunctionType.Exp,
                        scale=EXP_SCALE, bias=ebias)
                    if j >= 0:
                        # zero the k>q half of the diagonal tile in place
                        # (local col c vs partition p: keep iff c >= p)
                        nc.gpsimd.affine_select(
                            out=pr[:, x0:512], in_=pr[:, x0:512],
                            pattern=[[1, F]], base=0,
                            channel_multiplier=-1,
                            compare_op=mybir.AluOpType.is_ge, fill=0.0)
                    pend.append((h, pr, x0, kt))
                    if len(pend) > LAG:
                        consume()
                        for e in fin_q:
                            e[1] += 1
                        if fin_q and fin_q[0][1] >= 2:
                            finalize(fin_q.popleft()[0])
            while pend:
                consume()
            while fin_q:
                finalize(fin_q.popleft()[0])

        # ------- Fused phases: per token tile: projection+conv, then the
        # attention q-tile that just became computable, then the (pipelined)
        # output projection of the previous q-tile.
        for tt in range(NTT):
            ht_hi = p_ht.tile([P, NPR, 2, 512], FP8, tag="hth")
            ht_lo = p_ht.tile([P, NPR, 2, 512], FP8, tag="htl")
            if tt == 0:
                # weights stream on the ACT hwdge queue (idle at startup), in
                # the same per-fc order the projection chains consume them;
                # hidden-state chunks stream in parallel on the SP queue
                for fc in (4, 5, 0, 1, 2, 3):
                    nc.scalar.dma_start(wq_hi[:, fc], wh_d[:, fc])
                    nc.scalar.dma_start(wq_lo[:, fc], wl_d[:, fc])
            # tile-major host layout -> fully contiguous lines; two large
            # DMAs per tensor (dispatch cost is per-descriptor, so fewer,
            # bigger transfers beat fine-grained chunking)
            nchunk = 4 if tt == 0 else 2
            for c_ in range(nchunk):
                w_ = NPR // nchunk
                nc.sync.dma_start(ht_hi[:, ds(c_ * w_, w_)],
                                  hh_d[:, tt, ds(c_ * w_, w_)])
                nc.sync.dma_start(ht_lo[:, ds(c_ * w_, w_)],
                                  hl_d[:, tt, ds(c_ * w_, w_)])
            t0 = tt * 512

            def proj_group(pp, fc, g):
                lhs, rhs_h = ((wq_hi, ht_hi), (wq_hi, ht_lo),
                              (wq_lo, ht_hi))[g]
                for pr_ in range(NPR):
                    nc.tensor.matmul(
                        pp, lhsT=lhs[:, fc, pr_], rhs=rhs_h[:, pr_],
                        start=(g == 0 and pr_ == 0),
                        stop=(g == 2 and pr_ == NPR - 1), perf_mode=DR)

            def conv_fc(fc, pp):
                # pre-conv x (64x) -> bf16 for the DVE conv taps
                nc.scalar.copy(qkvf[:, fc, ds(3 + t0, 512)], pp)
                # conv taps: out[t] = x[t] + sum_k x[t+k-3]*w[k].
                # Products via tensor_scalar (4x DVE mode — the tensor-tensor
                # variant gets no fast mode), sums via bf16 tensor_tensor (2x)
                ca = p_work.tile([P, 512], BF16, tag="ctmpa", name="ca")
                cb = p_work.tile([P, 512], BF16, tag="ctmpb", name="cb")
                nc.vector.tensor_scalar(
                    ca, qkvf[:, fc, ds(t0 + 0, 512)],
                    cw[:, fc * 4 + 0: fc * 4 + 1], None, op0=MULT)
                nc.vector.tensor_scalar(
                    cb, qkvf[:, fc, ds(t0 + 1, 512)],
                    cw[:, fc * 4 + 1: fc * 4 + 2], None, op0=MULT)
                nc.vector.tensor_add(ca, ca, cb)
                nc.vector.tensor_scalar(
                    cb, qkvf[:, fc, ds(t0 + 2, 512)],
                    cw[:, fc * 4 + 2: fc * 4 + 3], None, op0=MULT)
                nc.vector.tensor_add(ca, ca, cb)
                nc.vector.tensor_scalar(
                    cb, qkvf[:, fc, ds(t0 + 3, 512)],
                    cw[:, fc * 4 + 3: fc * 4 + 4], None, op0=MULT)
                nc.vector.tensor_add(cb, cb, qkvf[:, fc, ds(t0 + 3, 512)])
                nc.vector.tensor_add(qkvb[:, fc, ts(tt, 512)], ca, cb)

            for fc in (4, 5, 0, 1, 2, 3):
                pp = ps2.tile([P, 512], F32, tag="proj", name="pp")
                for g in range(3):
                    proj_group(pp, fc, g)
                conv_fc(fc, pp)
            # v (fc=5) of this token tile -> natural [token, dh] layout
            trp = ps1.tile([P, 512], BF16, tag="small")
            for j in range(4):
                nc.tensor.transpose(trp[:, ds(j * P, P)],
                                    qkvb[:, 5, ds((tt * 4 + j) * P, P)],
                                    ident)
            nc.vector.tensor_copy(vnat[:, ds(tt * 4, 4), :], trp)
            if tt == 0:
                # w_o load deferred past the critical head DMAs
                nc.sync.dma_start(wo_hi, oh_d)
                nc.sync.dma_start(wo_lo, ol_d)
            attn_B(tt)
        for t4 in range(4):
            o_proj_chunk(NQT - 1, t4, final=True)
        if debug:
            nc.sync.dma_start(dbg_qkvb, qkvb)
            nc.sync.dma_start(dbg_atth, atth)
            nc.sync.dma_start(dbg_qkvf8, qkvf)

    if legalize:
        _legalize_waits(nc)
    _CACHE[key] = nc
    return nc


def _prep_inputs(hidden_states, w_q, w_k, w_v, w_o, conv_w):
    """Build the 8 per-core input maps (host-side shard + fp8 hi/lo split)."""
    f8 = ml_dtypes.float8_e4m3

    def hpairs(x):  # [2048 d, 2048 t] -> [128, 4, 8, 2, 512]
        return np.ascontiguousarray(
            x.reshape(NPR, 2, P, NTT, 512).transpose(2, 3, 0, 1, 4))

    def wpairs(x):  # [2048, 768] -> [128, 6, 8, 2, 128]
        return np.ascontiguousarray(
            x.reshape(NPR, 2, P, NF, P).transpose(2, 3, 0, 1, 4))

    def split8(x):
        hi = x.astype(f8)
        lo = (x - hi.astype(np.float32)).astype(f8)
        return hi, lo

    # hidden split is shared by the 4 cores of a batch
    h_pairs = []
    for b in range(2):
        hT = np.ascontiguousarray(hidden_states[b].T)
        hi, lo = split8(hT)
        h_pairs.append((hpairs(hi), hpairs(lo)))

    in_maps = []
    for c in range(8):
        b, g = c // 4, c % 4
        wqkv = np.concatenate(
            [w_q[:, g * 512:(g + 1) * 512],
             w_k[:, g * 128:(g + 1) * 128],
             w_v[:, g * 128:(g + 1) * 128]], axis=1) * WSCALE
        w_hi, w_lo = split8(wqkv)
        wo = np.ascontiguousarray(w_o[g * 512:(g + 1) * 512, :]) * WSCALE
        wo_hi, wo_lo = split8(wo)
        wo_hi = np.ascontiguousarray(
            wo_hi.reshape(2, 2, P, D).transpose(2, 0, 1, 3))
        wo_lo = np.ascontiguousarray(
            wo_lo.reshape(2, 2, P, D).transpose(2, 0, 1, 3))
        cw = np.concatenate(
            [conv_w[g * 512:(g + 1) * 512],
             conv_w[2048 + g * 128: 2048 + (g + 1) * 128],
             conv_w[2560 + g * 128: 2560 + (g + 1) * 128]], axis=0)  # [768,4]
        cwp = np.ascontiguousarray(
            cw.reshape(NF, P, 4).transpose(1, 0, 2).reshape(P, NF * 4)
        ).astype(np.float32)
        in_maps.append({
            "h_hi": h_pairs[b][0], "h_lo": h_pairs[b][1],
            "w_hi": wpairs(w_hi), "w_lo": wpairs(w_lo),
            "wo_hi": wo_hi, "wo_lo": wo_lo,
            "conv_w": cwp,
        })
    return in_maps


def kernel(hidden_states, w_q, w_k, w_v, w_o, conv_w, _trace=False):
    nc = _build()
    in_maps = _prep_inputs(
        np.asarray(hidden_states, dtype=np.float32),
        np.asarray(w_q, dtype=np.float32),
        np.asarray(w_k, dtype=np.float32),
        np.asarray(w_v, dtype=np.float32),
        np.asarray(w_o, dtype=np.float32),
        np.asarray(conv_w, dtype=np.float32),
    )
    res = run_bass_kernel_spmd(nc, in_maps, core_ids=list(range(8)),
                               trace=_trace)
    outs = [r["out"] for r in res.results]
    full = np.empty((2, S, D), dtype=np.float32)
    for b in range(2):
        acc = outs[4 * b] + outs[4 * b + 1] + outs[4 * b + 2] + outs[4 * b + 3]
        full[b] = acc * (1.0 / OUT_DIV)
    if _trace:
        kernel.last_results = res
    return full
